# revision 10
# baseline (speedup 1.0000x reference)
"""Trainium2 Bass kernel for nn_BottleneckVQa (VQ bottleneck autoencoder).

Data-parallel over 8 NeuronCores: batch 32 -> 4 images/core. Encoder convs run
in f32r (TF32-like, full PE rate), decoder convs in bf16. VQ argmin via
PE transpose + vector max/max_index; codes gathered by indirect DMA.

kernel(**inputs) takes the full unsharded inputs and returns
(x_hat, z_probs, vq_loss) matching the reference.
"""
import os
import sys
import numpy as np

sys.path.insert(0, "/opt/trn_rl_repo")

import ml_dtypes
import concourse.bass as bass
import concourse.tile as tile
from concourse import bacc, mybir
from concourse.bass_utils import run_bass_kernel_spmd
from concourse.masks import make_identity
from contextlib import ExitStack

F32 = mybir.dt.float32
F32R = mybir.dt.float32r
BF16 = mybir.dt.bfloat16
U32 = mybir.dt.uint32
GELU = mybir.ActivationFunctionType.Gelu if os.environ.get("KERNEL_SIM", "0") != "1" else mybir.ActivationFunctionType.Copy
COPY = mybir.ActivationFunctionType.Copy
IDENT = mybir.ActivationFunctionType.Identity

P_IMG = 4            # images per core
NCORES = 8
STAGES = int(os.environ.get("KERNEL_STAGES", "8"))
DEBUG = os.environ.get("KERNEL_DEBUG", "0") == "1"
SIM_SAFE = os.environ.get("KERNEL_SIM", "0") == "1"
NOZ2 = os.environ.get("KERNEL_NOZ2", "0") == "1"
RAW_F32R = False     # f32r dropped: encoder must be exact fp32 (argmin flips)

_CACHE = {}


# ---------------------------------------------------------------- device build
def build_nc():
    nc = bacc.Bacc("TRN2", target_bir_lowering=False, debug=False, num_devices=NCORES)

    dt_c1 = F32
    # inputs
    IM2R = nc.dram_tensor("im2r", [75, P_IMG * 16384], dt_c1, kind="ExternalInput").ap()
    W1P = nc.dram_tensor("w1p", [75, 64], dt_c1, kind="ExternalInput").ap()
    W2P = nc.dram_tensor("w2p", [2, 5, 128, 64], F32, kind="ExternalInput").ap()
    W2P4 = nc.dram_tensor("w2p4", [5, 64, 64], F32, kind="ExternalInput").ap()
    W3P = nc.dram_tensor("w3p", [4, 64, 64], F32, kind="ExternalInput").ap()
    WD1 = nc.dram_tensor("wd1", [2, 128, 512], BF16, kind="ExternalInput").ap()
    WD2 = nc.dram_tensor("wd2", [4, 4, 128, 256], BF16, kind="ExternalInput").ap()
    WD3 = nc.dram_tensor("wd3", [4, 2, 128, 256], BF16, kind="ExternalInput").ap()
    CODES2 = nc.dram_tensor("codes2", [64, 24], F32, kind="ExternalInput").ap()
    C2N = nc.dram_tensor("c2n", [24, 1], F32, kind="ExternalInput").ap()
    CTBL = nc.dram_tensor("ctbl", [24, 128], F32, kind="ExternalInput").ap()

    # outputs
    XH = nc.dram_tensor("xh", [P_IMG, 256, 4096], F32, kind="ExternalOutput").ap()
    IDXO = nc.dram_tensor("idxo", [P_IMG, 3969], U32, kind="ExternalOutput").ap()
    ZACC = nc.dram_tensor("zacc_o", [64, 32], F32, kind="ExternalOutput").ap()
    MACC = nc.dram_tensor("macc_o", [128, 128], F32, kind="ExternalOutput").ap()
    if DEBUG:
        DA1 = nc.dram_tensor("dbg_a1", [128, 66 * 132], F32, kind="ExternalOutput").ap()
        DA2 = nc.dram_tensor("dbg_a2", [64, 4096], F32, kind="ExternalOutput").ap()
        DZ = nc.dram_tensor("dbg_z", [64, 3969], F32, kind="ExternalOutput").ap()
        DS = nc.dram_tensor("dbg_ssb", [24, 3969], F32, kind="ExternalOutput").ap()
        DZQ = nc.dram_tensor("dbg_zqb", [128, 65 * 68], BF16, kind="ExternalOutput").ap()
        DY1 = nc.dram_tensor("dbg_y1", [128, 4096], BF16, kind="ExternalOutput").ap()
        DY2 = nc.dram_tensor("dbg_y2", [128, 65 * 68], BF16, kind="ExternalOutput").ap()

    with tile.TileContext(nc) as tc, ExitStack() as ctx:
        wpool = ctx.enter_context(tc.tile_pool(name="wpool", bufs=1))
        stg = ctx.enter_context(tc.tile_pool(name="stg", bufs=2))
        actp = ctx.enter_context(tc.tile_pool(name="actp", bufs=1))
        im2p = ctx.enter_context(tc.tile_pool(name="im2p", bufs=3))
        vqp = ctx.enter_context(tc.tile_pool(name="vqp", bufs=3))
        outp = ctx.enter_context(tc.tile_pool(name="outp", bufs=3))
        psum = ctx.enter_context(tc.tile_pool(name="psum", bufs=4, space="PSUM"))
        psvq = ctx.enter_context(tc.tile_pool(name="psvq", bufs=2, space="PSUM"))

        # ---- static setup: identities, weights, accumulators ----
        ident = wpool.tile([128, 128], F32, tag="ident")
        make_identity(nc, ident[:])

        w1p = wpool.tile([75, 64], dt_c1, tag="w1p")
        nc.sync.dma_start(w1p[:], W1P)

        w2p = wpool.tile([128, 10 * 64], F32, tag="w2p")
        for g in range(2):
            for kw in range(5):
                nc.sync.dma_start(w2p[:, (g * 5 + kw) * 64:(g * 5 + kw + 1) * 64], W2P[g, kw])
        w2p4 = wpool.tile([64, 5 * 64], F32, tag="w2p4")
        for kw in range(5):
            nc.sync.dma_start(w2p4[:, kw * 64:(kw + 1) * 64], W2P4[kw])
        w3p = wpool.tile([64, 4 * 64], F32, tag="w3p")
        for t_ in range(4):
            nc.sync.dma_start(w3p[:, t_ * 64:(t_ + 1) * 64], W3P[t_])
        codes2 = wpool.tile([64, 24], F32, tag="codes2")
        nc.sync.dma_start(codes2[:], CODES2)
        c2n = wpool.tile([24, 1], F32, tag="c2n")
        nc.sync.dma_start(c2n[:], C2N)

        wd1 = wpool.tile([128, 2 * 512], BF16, tag="wd1")
        for t_ in range(2):
            nc.sync.dma_start(wd1[:, t_ * 512:(t_ + 1) * 512], WD1[t_])
        wd2 = wpool.tile([128, 16 * 256], BF16, tag="wd2")
        for t_ in range(4):
            for s in range(4):
                nc.sync.dma_start(wd2[:, (t_ * 4 + s) * 256:(t_ * 4 + s + 1) * 256], WD2[t_, s])
        wd3 = wpool.tile([128, 8 * 256], BF16, tag="wd3")
        for t_ in range(4):
            for s in range(2):
                nc.sync.dma_start(wd3[:, (t_ * 2 + s) * 256:(t_ * 2 + s + 1) * 256], WD3[t_, s])

        macc = actp.tile([128, 128], F32, tag="macc")
        nc.gpsimd.memset(macc[:], 0.0)
        zacc = actp.tile([64, 32], F32, tag="zacc")
        nc.gpsimd.memset(zacc[:], 0.0)

        # persistent per-image activation buffers (bufs=1 -> reused across images)
        a1 = actp.tile([128, 66 * 132], F32, tag="a1")      # conv1 out, parity-interleaved, padded
        a2 = actp.tile([64, 64 * 64], F32, tag="a2")        # conv2 out, plain
        zt = actp.tile([64, 3969], F32, tag="zt")           # conv3 out (z)
        ssb = actp.tile([24, 3969], F32, tag="ssb")          # scores 2*z.c - c2
        zqb = actp.tile([128, 65 * 68], BF16, tag="zqb")     # zq dup-channel (hi half col-shifted by 1)
        y1 = [actp.tile([128, 4096], BF16, name=f"y1_{s}", tag=f"y1_{s}") for s in range(4)]
        y2 = [actp.tile([128, 65 * 68], BF16, name=f"y2_{s}", tag=f"y2_{s}") for s in range(2)]

        a1v = a1[:].rearrange("p (s c) -> p s c", s=66, c=132)
        # zero pad regions once: interiors are fully rewritten every image
        nc.gpsimd.memset(a1[:], 0.0)
        nc.gpsimd.memset(zqb[:], 0.0)
        for s_ in range(2):
            nc.gpsimd.memset(y2[s_][:], 0.0)

        for img in range(P_IMG):
            # ---------------- conv1: im2row matmul, K=75, M=128(dup) ----------------
            for rb in range(32):  # 4 output rows per block
                t = im2p.tile([75, 512], dt_c1, tag="im2t")
                nc.sync.dma_start(t[:], IM2R[:, img * 16384 + rb * 512: img * 16384 + (rb + 1) * 512])
                p = psum.tile([64, 512], F32, tag="mm")
                nc.tensor.matmul(p[:], w1p[:], t[:], start=True, stop=True)
                pv = p[:].rearrange("p (r c) -> p r c", r=4, c=128)
                # rows 4rb+0..3; slot = r//2 + 1; parity r%2
                nc.scalar.activation(a1v[0:64, 2 * rb + 1: 2 * rb + 3, 2:130], pv[0:64, 0:4:2, :], GELU)
                odd = stg.tile([64, 256], F32, tag="odd")
                nc.scalar.activation(odd[:].rearrange("p (r c) -> p r c", r=2, c=128), pv[0:64, 1:4:2, :], GELU)
                nc.sync.dma_start(a1v[64:128, 2 * rb + 1: 2 * rb + 3, 2:130], odd[:].rearrange("p (r c) -> p r c", r=2, c=128))

            if DEBUG and img == 0:
                nc.sync.dma_start(DA1, a1[:])
            if STAGES < 2:
                continue
            # ---------------- conv2: K=128 kh-pairs + K=64 tail, M=64 ----------------
            a2v = a2[:].rearrange("p (r c) -> p r c", r=64, c=64)
            for t8 in range(8):
                oh0 = 8 * t8
                p = psum.tile([64, 512], F32, tag="mm")
                first = True
                for g in range(2):
                    for kw in range(5):
                        nc.tensor.matmul(
                            p[:], w2p[:, (g * 5 + kw) * 64:(g * 5 + kw + 1) * 64],
                            a1v[:, oh0 + g: oh0 + g + 8, kw: kw + 128: 2],
                            start=first, stop=False)
                        first = False
                for kw in range(5):
                    nc.tensor.matmul(
                        p[:], w2p4[:, kw * 64:(kw + 1) * 64],
                        a1v[0:64, oh0 + 2: oh0 + 10, kw: kw + 128: 2],
                        start=False, stop=(kw == 4))
                nc.scalar.activation(a2v[:, oh0: oh0 + 8, :], p[:].rearrange("p (r c) -> p r c", r=8, c=64), GELU)

            if DEBUG and img == 0:
                nc.sync.dma_start(DA2, a2[:])
            if STAGES < 3:
                continue
            # ---------------- conv3: K=64, 4 taps, M=64 -> z ----------------
            for t8 in range(8):
                oh0 = 8 * t8
                nr = 8 if t8 < 7 else 7
                p = psum.tile([64, 504], F32, tag="mm")
                for tap in range(4):
                    kh, kw = tap // 2, tap % 2
                    nc.tensor.matmul(
                        p[:, :nr * 63], w3p[:, tap * 64:(tap + 1) * 64],
                        a2v[0:64, oh0 + kh: oh0 + kh + nr, kw: kw + 63],
                        start=(tap == 0), stop=(tap == 3))
                nc.scalar.activation(zt[:, oh0 * 63: (oh0 + nr) * 63], p[:, :nr * 63], COPY)
                if not NOZ2:
                    z2s = stg.tile([64, 504], F32, tag="z2s")
                    nc.vector.tensor_mul(z2s[:, :nr * 63], zt[:, oh0 * 63: (oh0 + nr) * 63],
                                         zt[:, oh0 * 63: (oh0 + nr) * 63])
                    nc.vector.tensor_reduce(
                        out=zacc[:, img * 8 + t8: img * 8 + t8 + 1],
                        in_=z2s[:, :nr * 63], op=mybir.AluOpType.add,
                        axis=mybir.AxisListType.X)

            if DEBUG and img == 0:
                nc.sync.dma_start(DZ, zt[:])
            if STAGES < 4:
                continue
            # ---------------- scores: s' = 2 z.c - c2 ----------------
            for c8 in range(8):
                n0 = c8 * 512
                n1 = min(n0 + 512, 3969)
                p = psum.tile([24, 512], F32, tag="mm")
                nc.tensor.matmul(p[:, :n1 - n0], codes2[:], zt[:, n0:n1], start=True, stop=True)
                nc.scalar.activation(ssb[:, n0:n1], p[:, :n1 - n0], IDENT, bias=c2n[:])

            if DEBUG and img == 0:
                nc.sync.dma_start(DS, ssb[:])
            if STAGES < 5:
                continue
            # ---------------- VQ: transpose -> max/argmax -> gather -> zq ----------------
            zqv = zqb[:].rearrange("p (r c) -> p r c", r=65, c=68)
            for ch in range(32):
                n0 = ch * 126
                n = 126 if ch < 31 else 63
                pT = psvq.tile([126, 24], F32, tag="pT")
                nc.tensor.transpose(pT[:n, :], ssb[:, n0:n0 + n], ident[0:24, 0:24])
                sT = vqp.tile([126, 24], F32, tag="sT")
                nc.vector.tensor_copy(sT[:n, :], pT[:n, :])
                mxt = vqp.tile([126, 8], F32, tag="mxt")
                nc.vector.max(mxt[:n, :], sT[:n, :])
                mi = vqp.tile([126, 8], U32, tag="mi")
                nc.vector.max_index(mi[:n, :], mxt[:n, :], sT[:n, :])
                nc.vector.tensor_copy(macc[0:n, img * 32 + ch: img * 32 + ch + 1], mxt[0:n, 0:1])
                nc.sync.dma_start(IDXO[img, n0:n0 + n], mi[0:n, 0:1])
                zqT = vqp.tile([126, 128], F32, tag="zqT")
                nc.gpsimd.indirect_dma_start(
                    out=zqT[:n, :], out_offset=None, in_=CTBL,
                    in_offset=bass.IndirectOffsetOnAxis(ap=mi[0:n, 0:1], axis=0))
                pq = psvq.tile([128, 126], F32, tag="pq")
                nc.tensor.transpose(pq[:, :n], zqT[:n, :], ident[0:n, 0:n])
                # rows 2ch, 2ch+1 -> padded rows 2ch+1, 2ch+2; hi half shifted left 1 col
                nc.scalar.activation(
                    zqv[0:64, 2 * ch + 1: 2 * ch + 1 + n // 63, 1:64],
                    pq[0:64, :n].rearrange("p (r c) -> p r c", r=n // 63, c=63), COPY)
                nc.scalar.activation(
                    zqv[64:128, 2 * ch + 1: 2 * ch + 1 + n // 63, 0:63],
                    pq[64:128, :n].rearrange("p (r c) -> p r c", r=n // 63, c=63), COPY)

            if DEBUG and img == 0:
                nc.sync.dma_start(DZQ, zqb[:])
            if STAGES < 6:
                continue
            # ---------------- dec1: K=64, 4 taps, M=512 (4 chunks) ----------------
            for t8 in range(8):
                oh0 = 8 * t8
                for mch in range(4):
                    p = psum.tile([128, 512], F32, tag="mm")
                    for kh in range(2):
                        nc.tensor.matmul(
                            p[:], wd1[:, kh * 512 + mch * 128: kh * 512 + (mch + 1) * 128],
                            zqv[:, oh0 + kh: oh0 + kh + 8, 0:64],
                            start=(kh == 0), stop=(kh == 1))
                    nc.scalar.activation(y1[mch][:, oh0 * 64: (oh0 + 8) * 64], p[:], GELU)

            if DEBUG and img == 0:
                nc.sync.dma_start(DY1, y1[0][:])
            if STAGES < 7:
                continue
            # ---------------- dec2: K=512 (4 slabs), 4 taps, M=256 (2 chunks) ----------------
            y1v = [y1[s][:].rearrange("p (r c) -> p r c", r=64, c=64) for s in range(4)]
            y2v = [y2[s][:].rearrange("p (r c) -> p r c", r=65, c=68) for s in range(2)]
            for t8 in range(8):
                oh0 = 8 * t8
                nr = 8 if t8 < 7 else 7
                for mch in range(2):
                    p = psum.tile([128, 504], F32, tag="mm")
                    first = True
                    for tap in range(4):
                        kh, kw = tap // 2, tap % 2
                        for s in range(4):
                            nc.tensor.matmul(
                                p[:, :nr * 63],
                                wd2[:, (tap * 4 + s) * 256 + mch * 128: (tap * 4 + s) * 256 + (mch + 1) * 128],
                                y1v[s][:, oh0 + kh: oh0 + kh + nr, kw: kw + 63],
                                start=first, stop=(tap == 3 and s == 3))
                            first = False
                    nc.scalar.activation(
                        y2v[mch][:, oh0 + 1: oh0 + 1 + nr, 1:64],
                        p[:, :nr * 63].rearrange("p (r c) -> p r c", r=nr, c=63), GELU)

            if DEBUG and img == 0:
                nc.sync.dma_start(DY2, y2[0][:])
            if STAGES < 8:
                continue
            # ---------------- dec3: K=256 (2 slabs), 4 taps, M=256 (2 chunks) ----------------
            for t8 in range(8):
                oh0 = 8 * t8
                for mch in range(2):
                    p = psum.tile([128, 512], F32, tag="mm")
                    first = True
                    for tap in range(4):
                        kh, kw = tap // 2, tap % 2
                        for s in range(2):
                            nc.tensor.matmul(
                                p[:],
                                wd3[:, (tap * 2 + s) * 256 + mch * 128: (tap * 2 + s) * 256 + (mch + 1) * 128],
                                y2v[s][:, oh0 + kh: oh0 + kh + 8, kw: kw + 64],
                                start=first, stop=(tap == 3 and s == 1))
                            first = False
                    xo = outp.tile([128, 512], F32, tag="xo")
                    nc.scalar.activation(xo[:], p[:], COPY)
                    nc.sync.dma_start(XH[img, mch * 128:(mch + 1) * 128, oh0 * 64:(oh0 + 8) * 64], xo[:])

        nc.sync.dma_start(MACC, macc[:])
        nc.sync.dma_start(ZACC, zacc[:])

    nc.compile()
    return nc

# memset y2 pads once per image is needed: y2 pad cols/rows written never read?
# dec3 reads padded region -> must be zero. y2 buffers are reused across images;
# interior is fully overwritten each image, pads stay zero from a single memset.


def _setup_profiling():
    """Shim antenv.axon_hooks (absent on this image) with the boot module's
    ctypes NTFF hook, and neuter the artifact upload."""
    try:
        import types
        import concourse.bass_utils as bu
        from trn_agent_boot.trn_boot import _ntff_profile_via_ctypes
        import antenv
        if "antenv.axon_hooks" not in sys.modules:
            hook = _ntff_profile_via_ctypes("/opt/axon/libaxon_pjrt.so")
            if hook is None:
                return False
            mod = types.ModuleType("antenv.axon_hooks")
            mod.get_axon_ntff_profile_hook = lambda: hook
            mod.set_axon_ntff_profile_hook = lambda h: None
            sys.modules["antenv.axon_hooks"] = mod
            antenv.axon_hooks = mod
        bu.upload_artifacts = lambda tmpdir: "local://" + str(tmpdir)
        return True
    except Exception as e:  # pragma: no cover
        print(f"profiling setup failed: {e}", flush=True)
        return False


# ---------------------------------------------------------------- host packing
def _pack_weights(enc_w1, enc_w2, enc_w3, dec_w1, dec_w2, dec_w3, codes):
    bf = ml_dtypes.bfloat16
    w1p = enc_w1.transpose(2, 3, 1, 0).reshape(75, 64).astype(np.float32)

    w2p = np.zeros((2, 5, 128, 64), np.float32)
    for g in range(2):
        for kw in range(5):
            w2p[g, kw, 0:64] = enc_w2[:, :, 2 * g, kw].T
            w2p[g, kw, 64:128] = enc_w2[:, :, 2 * g + 1, kw].T
    w2p4 = np.stack([enc_w2[:, :, 4, kw].T for kw in range(5)], 0).astype(np.float32)
    w3p = np.stack([enc_w3[:, :, t // 2, t % 2].T for t in range(4)], 0).astype(np.float32)

    # dec1 kw-pair: lhsT[kh] rows 0:64 = w(kh,kw=0), rows 64:128 = w(kh,kw=1)
    wd1 = np.stack(
        [np.concatenate([dec_w1[:, :, kh, 0].T, dec_w1[:, :, kh, 1].T], axis=0)
         for kh in range(2)], 0).astype(bf)
    wd2 = np.stack(
        [np.stack([dec_w2[:, 128 * s:128 * (s + 1), t // 2, t % 2].T for s in range(4)], 0)
         for t in range(4)], 0).astype(bf)
    wd3 = np.stack(
        [np.stack([dec_w3[:, 128 * s:128 * (s + 1), t // 2, t % 2].T for s in range(2)], 0)
         for t in range(4)], 0).astype(bf)

    codes2 = (2.0 * codes.T).astype(np.float32)               # [64, 24]
    c2n = (-(codes.astype(np.float64) ** 2).sum(1)).astype(np.float32).reshape(24, 1)
    ctbl = np.concatenate([codes, codes], axis=1).astype(np.float32)   # [24, 128] dup
    return dict(w1p=w1p, w2p=w2p, w2p4=w2p4, w3p=w3p, wd1=wd1, wd2=wd2, wd3=wd3,
                codes2=codes2, c2n=c2n, ctbl=ctbl)


def _im2row(x4):
    """x4: [4, 3, 256, 256] fp32 -> [75, 4*16384] fp32 (conv1 im2row, pad 2 stride 2)."""
    xp = np.pad(x4, ((0, 0), (0, 0), (2, 2), (2, 2)))
    sl = np.empty((25, 4, 3, 128, 128), np.float32)
    for kh in range(5):
        for kw in range(5):
            sl[kh * 5 + kw] = xp[:, :, kh:kh + 256:2, kw:kw + 256:2]
    return sl.transpose(0, 2, 1, 3, 4).reshape(75, 4 * 16384)


# ---------------------------------------------------------------- entry point
def kernel(x, enc_w1, enc_w2, enc_w3, dec_w1, dec_w2, dec_w3, codes, ema_count):
    x = np.asarray(x, np.float32)
    wpack = _pack_weights(np.asarray(enc_w1), np.asarray(enc_w2), np.asarray(enc_w3),
                          np.asarray(dec_w1), np.asarray(dec_w2), np.asarray(dec_w3),
                          np.asarray(codes))

    key = (STAGES, DEBUG, NOZ2)
    if key not in _CACHE:
        _CACHE[key] = build_nc()
    nc = _CACHE[key]

    in_maps = []
    for core in range(NCORES):
        m = dict(wpack)
        m["im2r"] = _im2row(x[core * P_IMG:(core + 1) * P_IMG])
        in_maps.append(m)

    trace = os.environ.get("KERNEL_PROFILE", "0") == "1"
    if trace:
        trace = _setup_profiling()
    res = run_bass_kernel_spmd(nc, in_maps, list(range(NCORES)), trace=trace)
    _CACHE["res"] = res
    if trace and res.exec_time_ns is not None:
        print(f"HW exec time: {res.exec_time_ns} ns", flush=True)
        _CACHE["exec_time_ns"] = res.exec_time_ns

    B, C, H, W = 32, 64, 63, 63
    x_hat = np.empty((32, 256, 64, 64), np.float32)
    idx_full = np.empty((32, 3969), np.int64)
    z2_sum = 0.0
    smax_sum = 0.0
    for core in range(NCORES):
        r = res.results[core]
        x_hat[core * P_IMG:(core + 1) * P_IMG] = r["xh"].reshape(P_IMG, 256, 64, 64)
        idx_full[core * P_IMG:(core + 1) * P_IMG] = r["idxo"].astype(np.int64)
        z2_sum += float(r["zacc_o"].astype(np.float64).sum())
        smax_sum += float(r["macc_o"].astype(np.float64).sum())

    ema = np.asarray(ema_count, np.float64)
    probs = (ema / ema.sum()).astype(np.float32)
    z_probs = probs[idx_full].reshape(32, 63, 63).astype(np.float32)

    vq_loss = np.float32(0.25 * (z2_sum - smax_sum) / (B * C * H * W))
    return x_hat, z_probs, vq_loss


# revision 11
# speedup vs baseline: 1.1463x; 1.1463x over previous
"""Trainium2 Bass kernel for nn_BottleneckVQa (VQ bottleneck autoencoder).

Data-parallel over 8 NeuronCores: batch 32 -> 4 images/core. Encoder convs run
in f32r (TF32-like, full PE rate), decoder convs in bf16. VQ argmin via
PE transpose + vector max/max_index; codes gathered by indirect DMA.

kernel(**inputs) takes the full unsharded inputs and returns
(x_hat, z_probs, vq_loss) matching the reference.
"""
import os
import sys
import numpy as np

sys.path.insert(0, "/opt/trn_rl_repo")

import ml_dtypes
import concourse.bass as bass
import concourse.tile as tile
from concourse import bacc, mybir
from concourse.bass_utils import run_bass_kernel_spmd
from concourse.masks import make_identity
from contextlib import ExitStack

F32 = mybir.dt.float32
F32R = mybir.dt.float32r
BF16 = mybir.dt.bfloat16
U32 = mybir.dt.uint32
GELU = mybir.ActivationFunctionType.Gelu if os.environ.get("KERNEL_SIM", "0") != "1" else mybir.ActivationFunctionType.Copy
COPY = mybir.ActivationFunctionType.Copy
IDENT = mybir.ActivationFunctionType.Identity

P_IMG = 4            # images per core
NCORES = 8
STAGES = int(os.environ.get("KERNEL_STAGES", "8"))
DEBUG = os.environ.get("KERNEL_DEBUG", "0") == "1"
SIM_SAFE = os.environ.get("KERNEL_SIM", "0") == "1"
NOZ2 = os.environ.get("KERNEL_NOZ2", "0") == "1"
RAW_F32R = False     # f32r dropped: encoder must be exact fp32 (argmin flips)

_CACHE = {}


# ---------------------------------------------------------------- device build
def build_nc():
    nc = bacc.Bacc("TRN2", target_bir_lowering=False, debug=False, num_devices=NCORES)

    dt_c1 = F32
    # inputs
    IM2R = nc.dram_tensor("im2r", [128, P_IMG * 16384], dt_c1, kind="ExternalInput").ap()
    W1P = nc.dram_tensor("w1p", [128, 128], dt_c1, kind="ExternalInput").ap()
    W2P = nc.dram_tensor("w2p", [2, 5, 128, 64], F32, kind="ExternalInput").ap()
    W2P4 = nc.dram_tensor("w2p4", [5, 64, 64], F32, kind="ExternalInput").ap()
    W3P = nc.dram_tensor("w3p", [4, 64, 64], F32, kind="ExternalInput").ap()
    WD1 = nc.dram_tensor("wd1", [2, 128, 512], BF16, kind="ExternalInput").ap()
    WD2 = nc.dram_tensor("wd2", [4, 4, 128, 256], BF16, kind="ExternalInput").ap()
    WD3 = nc.dram_tensor("wd3", [4, 2, 128, 256], BF16, kind="ExternalInput").ap()
    CODES2 = nc.dram_tensor("codes2", [65, 24], F32, kind="ExternalInput").ap()
    CTBL = nc.dram_tensor("ctbl", [24, 128], F32, kind="ExternalInput").ap()

    # outputs
    XH = nc.dram_tensor("xh", [P_IMG, 256, 4096], F32, kind="ExternalOutput").ap()
    IDXO = nc.dram_tensor("idxo", [P_IMG, 3969], U32, kind="ExternalOutput").ap()
    ZACC = nc.dram_tensor("zacc_o", [64, 32], F32, kind="ExternalOutput").ap()
    MACC = nc.dram_tensor("macc_o", [128, 128], F32, kind="ExternalOutput").ap()
    if DEBUG:
        DA1 = nc.dram_tensor("dbg_a1", [128, 66 * 132], F32, kind="ExternalOutput").ap()
        DA2 = nc.dram_tensor("dbg_a2", [64, 4096], F32, kind="ExternalOutput").ap()
        DZ = nc.dram_tensor("dbg_z", [64, 3969], F32, kind="ExternalOutput").ap()
        DS = nc.dram_tensor("dbg_ssb", [24, 3969], F32, kind="ExternalOutput").ap()
        DZQ = nc.dram_tensor("dbg_zqb", [128, 65 * 68], BF16, kind="ExternalOutput").ap()
        DY1 = nc.dram_tensor("dbg_y1", [128, 4096], BF16, kind="ExternalOutput").ap()
        DY2 = nc.dram_tensor("dbg_y2", [128, 65 * 68], BF16, kind="ExternalOutput").ap()

    with tile.TileContext(nc) as tc, ExitStack() as ctx:
        wpool = ctx.enter_context(tc.tile_pool(name="wpool", bufs=1))
        stg = ctx.enter_context(tc.tile_pool(name="stg", bufs=2))
        actp = ctx.enter_context(tc.tile_pool(name="actp", bufs=1))
        im2p = ctx.enter_context(tc.tile_pool(name="im2p", bufs=3))
        vqp = ctx.enter_context(tc.tile_pool(name="vqp", bufs=6))
        outp = ctx.enter_context(tc.tile_pool(name="outp", bufs=3))
        psum = ctx.enter_context(tc.tile_pool(name="psum", bufs=3, space="PSUM"))
        psvq = ctx.enter_context(tc.tile_pool(name="psvq", bufs=3, space="PSUM"))

        # ---- static setup: identities, weights, accumulators ----
        ident = wpool.tile([128, 128], F32, tag="ident")
        make_identity(nc, ident[:])

        w1p = wpool.tile([128, 128], dt_c1, tag="w1p")
        nc.sync.dma_start(w1p[:], W1P)

        w2p = wpool.tile([128, 10 * 64], F32, tag="w2p")
        for g in range(2):
            for kw in range(5):
                nc.sync.dma_start(w2p[:, (g * 5 + kw) * 64:(g * 5 + kw + 1) * 64], W2P[g, kw])
        w2p4 = wpool.tile([64, 5 * 64], F32, tag="w2p4")
        for kw in range(5):
            nc.sync.dma_start(w2p4[:, kw * 64:(kw + 1) * 64], W2P4[kw])
        w3p = wpool.tile([64, 4 * 64], F32, tag="w3p")
        for t_ in range(4):
            nc.sync.dma_start(w3p[:, t_ * 64:(t_ + 1) * 64], W3P[t_])
        codes2 = wpool.tile([65, 24], F32, tag="codes2")
        nc.sync.dma_start(codes2[:], CODES2)

        wd1 = wpool.tile([128, 2 * 512], BF16, tag="wd1")
        for t_ in range(2):
            nc.sync.dma_start(wd1[:, t_ * 512:(t_ + 1) * 512], WD1[t_])
        wd2 = wpool.tile([128, 16 * 256], BF16, tag="wd2")
        for t_ in range(4):
            for s in range(4):
                nc.sync.dma_start(wd2[:, (t_ * 4 + s) * 256:(t_ * 4 + s + 1) * 256], WD2[t_, s])
        wd3 = wpool.tile([128, 8 * 256], BF16, tag="wd3")
        for t_ in range(4):
            for s in range(2):
                nc.sync.dma_start(wd3[:, (t_ * 2 + s) * 256:(t_ * 2 + s + 1) * 256], WD3[t_, s])

        macc = actp.tile([128, 128], F32, tag="macc")
        nc.gpsimd.memset(macc[:], 0.0)
        zacc = actp.tile([64, 32], F32, tag="zacc")
        nc.gpsimd.memset(zacc[:], 0.0)

        # persistent per-image activation buffers (bufs=1 -> reused across images)
        a1 = actp.tile([128, 66 * 132], F32, tag="a1")      # conv1 out, parity-interleaved, padded
        a2 = actp.tile([64, 64 * 64], F32, tag="a2")        # conv2 out, plain
        zt = actp.tile([65, 3969], F32, tag="zt")           # conv3 out (z); row 64 = ones
        zqb_ab = [actp.tile([128, 65 * 68], BF16, name=f"zqb{i}", tag=f"zqb{i}") for i in range(2)]
        y1 = [actp.tile([128, 4096], BF16, name=f"y1_{s}", tag=f"y1_{s}") for s in range(4)]
        y2 = [actp.tile([128, 65 * 68], BF16, name=f"y2_{s}", tag=f"y2_{s}") for s in range(2)]

        a1v = a1[:].rearrange("p (s c) -> p s c", s=66, c=132)
        # zero pad regions once: interiors are fully rewritten every image
        nc.gpsimd.memset(a1[:], 0.0)
        nc.gpsimd.memset(zqb_ab[0][:], 0.0)
        nc.gpsimd.memset(zqb_ab[1][:], 0.0)
        nc.gpsimd.memset(zt[64:65, :], 1.0)
        for s_ in range(2):
            nc.gpsimd.memset(y2[s_][:], 0.0)

        for img in range(P_IMG):
            # ---------------- conv1: im2row matmul, K=75, M=128(dup) ----------------
            for rb in range(32):  # 4 output rows per block
                t = im2p.tile([128, 512], dt_c1, tag="im2t")
                nc.sync.dma_start(t[:], IM2R[:, img * 16384 + rb * 512: img * 16384 + (rb + 1) * 512])
                p = psum.tile([128, 512], F32, tag="mm")
                nc.tensor.matmul(p[:], w1p[:], t[:], start=True, stop=True)
                pv = p[:].rearrange("p (r c) -> p r c", r=4, c=128)
                # rows 4rb+0..3; slot = r//2 + 1; parity r%2 (M duplicated across halves)
                nc.scalar.activation(a1v[0:64, 2 * rb + 1: 2 * rb + 3, 2:130], pv[0:64, 0:4:2, :], GELU)
                nc.scalar.activation(a1v[64:128, 2 * rb + 1: 2 * rb + 3, 2:130], pv[64:128, 1:4:2, :], GELU)

            if DEBUG and img == 0:
                nc.sync.dma_start(DA1, a1[:])
            if STAGES < 2:
                continue
            # ---------------- conv2: K=128 kh-pairs + K=64 tail, M=64 ----------------
            a2v = a2[:].rearrange("p (r c) -> p r c", r=64, c=64)
            for t8 in range(8):
                oh0 = 8 * t8
                p = psum.tile([64, 512], F32, tag="mm")
                first = True
                for g in range(2):
                    for kw in range(5):
                        nc.tensor.matmul(
                            p[:], w2p[:, (g * 5 + kw) * 64:(g * 5 + kw + 1) * 64],
                            a1v[:, oh0 + g: oh0 + g + 8, kw: kw + 128: 2],
                            start=first, stop=False)
                        first = False
                for kw in range(5):
                    nc.tensor.matmul(
                        p[:], w2p4[:, kw * 64:(kw + 1) * 64],
                        a1v[0:64, oh0 + 2: oh0 + 10, kw: kw + 128: 2],
                        start=False, stop=(kw == 4))
                nc.scalar.activation(a2v[:, oh0: oh0 + 8, :], p[:].rearrange("p (r c) -> p r c", r=8, c=64), GELU)

            if DEBUG and img == 0:
                nc.sync.dma_start(DA2, a2[:])
            if STAGES < 3:
                continue
            # ---------------- conv3: K=64, 4 taps, M=64 -> z ----------------
            for t8 in range(8):
                oh0 = 8 * t8
                nr = 8 if t8 < 7 else 7
                p = psum.tile([64, 504], F32, tag="mm")
                for tap in range(4):
                    kh, kw = tap // 2, tap % 2
                    nc.tensor.matmul(
                        p[:, :nr * 63], w3p[:, tap * 64:(tap + 1) * 64],
                        a2v[0:64, oh0 + kh: oh0 + kh + nr, kw: kw + 63],
                        start=(tap == 0), stop=(tap == 3))
                nc.scalar.activation(zt[0:64, oh0 * 63: (oh0 + nr) * 63], p[:, :nr * 63], COPY)
                if not NOZ2:
                    z2s = stg.tile([64, 504], F32, tag="z2s")
                    nc.vector.tensor_mul(z2s[:, :nr * 63], zt[0:64, oh0 * 63: (oh0 + nr) * 63],
                                         zt[0:64, oh0 * 63: (oh0 + nr) * 63])
                    nc.vector.tensor_reduce(
                        out=zacc[:, img * 8 + t8: img * 8 + t8 + 1],
                        in_=z2s[:, :nr * 63], op=mybir.AluOpType.add,
                        axis=mybir.AxisListType.X)

            if DEBUG and img == 0:
                nc.sync.dma_start(DZ, zt[0:64, :])
            if STAGES < 5:
                continue
            # ---------------- VQ: scores-mm -> max/argmax -> gather -> zq ----------------
            zqb = zqb_ab[img % 2]
            zqv = zqb[:].rearrange("p (r c) -> p r c", r=65, c=68)
            for ch in range(32):
                n0 = ch * 126
                n = 126 if ch < 31 else 63
                pT = psvq.tile([126, 24], F32, tag="pT")
                nc.tensor.matmul(pT[:n, :], zt[:, n0:n0 + n], codes2[:], start=True, stop=True)
                sT = vqp.tile([126, 24], F32, tag="sT")
                nc.vector.tensor_copy(sT[:n, :], pT[:n, :])
                mxt = vqp.tile([126, 8], F32, tag="mxt")
                nc.vector.max(mxt[:n, :], sT[:n, :])
                mi = vqp.tile([126, 8], U32, tag="mi")
                nc.vector.max_index(mi[:n, :], mxt[:n, :], sT[:n, :])
                nc.vector.tensor_copy(macc[0:n, img * 32 + ch: img * 32 + ch + 1], mxt[0:n, 0:1])
                nc.sync.dma_start(IDXO[img, n0:n0 + n], mi[0:n, 0:1])
                zqT = vqp.tile([126, 128], F32, tag="zqT")
                nc.gpsimd.indirect_dma_start(
                    out=zqT[:n, :], out_offset=None, in_=CTBL,
                    in_offset=bass.IndirectOffsetOnAxis(ap=mi[0:n, 0:1], axis=0))
                pq = psvq.tile([128, 126], F32, tag="pq", bufs=2)
                nc.tensor.transpose(pq[:, :n], zqT[:n, :], ident[0:n, 0:n])
                # rows 2ch, 2ch+1 -> padded rows 2ch+1, 2ch+2; hi half shifted left 1 col
                nc.scalar.activation(
                    zqv[0:64, 2 * ch + 1: 2 * ch + 1 + n // 63, 1:64],
                    pq[0:64, :n].rearrange("p (r c) -> p r c", r=n // 63, c=63), COPY)
                nc.scalar.activation(
                    zqv[64:128, 2 * ch + 1: 2 * ch + 1 + n // 63, 0:63],
                    pq[64:128, :n].rearrange("p (r c) -> p r c", r=n // 63, c=63), COPY)

            if DEBUG and img == 0:
                nc.sync.dma_start(DZQ, zqb[:])
            if STAGES < 6:
                continue
            # ---------------- dec1: K=64, 4 taps, M=512 (4 chunks) ----------------
            for t8 in range(8):
                oh0 = 8 * t8
                for mch in range(4):
                    p = psum.tile([128, 512], F32, tag="mm")
                    for kh in range(2):
                        nc.tensor.matmul(
                            p[:], wd1[:, kh * 512 + mch * 128: kh * 512 + (mch + 1) * 128],
                            zqv[:, oh0 + kh: oh0 + kh + 8, 0:64],
                            start=(kh == 0), stop=(kh == 1))
                    nc.scalar.activation(y1[mch][:, oh0 * 64: (oh0 + 8) * 64], p[:], GELU)

            if DEBUG and img == 0:
                nc.sync.dma_start(DY1, y1[0][:])
            if STAGES < 7:
                continue
            # ---------------- dec2: K=512 (4 slabs), 4 taps, M=256 (2 chunks) ----------------
            y1v = [y1[s][:].rearrange("p (r c) -> p r c", r=64, c=64) for s in range(4)]
            y2v = [y2[s][:].rearrange("p (r c) -> p r c", r=65, c=68) for s in range(2)]
            for t8 in range(8):
                oh0 = 8 * t8
                nr = 8 if t8 < 7 else 7
                for mch in range(2):
                    p = psum.tile([128, 504], F32, tag="mm")
                    first = True
                    for tap in range(4):
                        kh, kw = tap // 2, tap % 2
                        for s in range(4):
                            nc.tensor.matmul(
                                p[:, :nr * 63],
                                wd2[:, (tap * 4 + s) * 256 + mch * 128: (tap * 4 + s) * 256 + (mch + 1) * 128],
                                y1v[s][:, oh0 + kh: oh0 + kh + nr, kw: kw + 63],
                                start=first, stop=(tap == 3 and s == 3))
                            first = False
                    nc.scalar.activation(
                        y2v[mch][:, oh0 + 1: oh0 + 1 + nr, 1:64],
                        p[:, :nr * 63].rearrange("p (r c) -> p r c", r=nr, c=63), GELU)

            if DEBUG and img == 0:
                nc.sync.dma_start(DY2, y2[0][:])
            if STAGES < 8:
                continue
            # ---------------- dec3: K=256 (2 slabs), 4 taps, M=256 (2 chunks) ----------------
            for t8 in range(8):
                oh0 = 8 * t8
                for mch in range(2):
                    p = psum.tile([128, 512], F32, tag="mm")
                    first = True
                    for tap in range(4):
                        kh, kw = tap // 2, tap % 2
                        for s in range(2):
                            nc.tensor.matmul(
                                p[:],
                                wd3[:, (tap * 2 + s) * 256 + mch * 128: (tap * 2 + s) * 256 + (mch + 1) * 128],
                                y2v[s][:, oh0 + kh: oh0 + kh + 8, kw: kw + 64],
                                start=first, stop=(tap == 3 and s == 1))
                            first = False
                    xo = outp.tile([128, 512], F32, tag="xo")
                    nc.scalar.activation(xo[:], p[:], COPY)
                    nc.sync.dma_start(XH[img, mch * 128:(mch + 1) * 128, oh0 * 64:(oh0 + 8) * 64], xo[:])

        nc.sync.dma_start(MACC, macc[:])
        nc.sync.dma_start(ZACC, zacc[:])

    nc.compile()
    return nc

# memset y2 pads once per image is needed: y2 pad cols/rows written never read?
# dec3 reads padded region -> must be zero. y2 buffers are reused across images;
# interior is fully overwritten each image, pads stay zero from a single memset.


def _setup_profiling():
    """Shim antenv.axon_hooks (absent on this image) with the boot module's
    ctypes NTFF hook, and neuter the artifact upload."""
    try:
        import types
        import concourse.bass_utils as bu
        from trn_agent_boot.trn_boot import _ntff_profile_via_ctypes
        import antenv
        if "antenv.axon_hooks" not in sys.modules:
            hook = _ntff_profile_via_ctypes("/opt/axon/libaxon_pjrt.so")
            if hook is None:
                return False
            mod = types.ModuleType("antenv.axon_hooks")
            mod.get_axon_ntff_profile_hook = lambda: hook
            mod.set_axon_ntff_profile_hook = lambda h: None
            sys.modules["antenv.axon_hooks"] = mod
            antenv.axon_hooks = mod
        bu.upload_artifacts = lambda tmpdir: "local://" + str(tmpdir)
        return True
    except Exception as e:  # pragma: no cover
        print(f"profiling setup failed: {e}", flush=True)
        return False


# ---------------------------------------------------------------- host packing
def _pack_weights(enc_w1, enc_w2, enc_w3, dec_w1, dec_w2, dec_w3, codes):
    bf = ml_dtypes.bfloat16
    w1im = enc_w1.transpose(2, 3, 1, 0).reshape(75, 64).astype(np.float32)
    w1p = np.zeros((128, 128), np.float32)
    w1p[:75, 0:64] = w1im
    w1p[:75, 64:128] = w1im

    w2p = np.zeros((2, 5, 128, 64), np.float32)
    for g in range(2):
        for kw in range(5):
            w2p[g, kw, 0:64] = enc_w2[:, :, 2 * g, kw].T
            w2p[g, kw, 64:128] = enc_w2[:, :, 2 * g + 1, kw].T
    w2p4 = np.stack([enc_w2[:, :, 4, kw].T for kw in range(5)], 0).astype(np.float32)
    w3p = np.stack([enc_w3[:, :, t // 2, t % 2].T for t in range(4)], 0).astype(np.float32)

    # dec1 kw-pair: lhsT[kh] rows 0:64 = w(kh,kw=0), rows 64:128 = w(kh,kw=1)
    wd1 = np.stack(
        [np.concatenate([dec_w1[:, :, kh, 0].T, dec_w1[:, :, kh, 1].T], axis=0)
         for kh in range(2)], 0).astype(bf)
    wd2 = np.stack(
        [np.stack([dec_w2[:, 128 * s:128 * (s + 1), t // 2, t % 2].T for s in range(4)], 0)
         for t in range(4)], 0).astype(bf)
    wd3 = np.stack(
        [np.stack([dec_w3[:, 128 * s:128 * (s + 1), t // 2, t % 2].T for s in range(2)], 0)
         for t in range(4)], 0).astype(bf)

    codes2 = np.concatenate(
        [(2.0 * codes.T), -(codes.astype(np.float64) ** 2).sum(1)[None, :]],
        axis=0).astype(np.float32)                             # [65, 24]
    ctbl = np.concatenate([codes, codes], axis=1).astype(np.float32)   # [24, 128] dup
    return dict(w1p=w1p, w2p=w2p, w2p4=w2p4, w3p=w3p, wd1=wd1, wd2=wd2, wd3=wd3,
                codes2=codes2, ctbl=ctbl)


def _im2row(x4):
    """x4: [4, 3, 256, 256] fp32 -> [75, 4*16384] fp32 (conv1 im2row, pad 2 stride 2)."""
    xp = np.pad(x4, ((0, 0), (0, 0), (2, 2), (2, 2)))
    sl = np.empty((25, 4, 3, 128, 128), np.float32)
    for kh in range(5):
        for kw in range(5):
            sl[kh * 5 + kw] = xp[:, :, kh:kh + 256:2, kw:kw + 256:2]
    out = np.zeros((128, 4 * 16384), np.float32)
    out[:75] = sl.transpose(0, 2, 1, 3, 4).reshape(75, 4 * 16384)
    return out


# ---------------------------------------------------------------- entry point
def kernel(x, enc_w1, enc_w2, enc_w3, dec_w1, dec_w2, dec_w3, codes, ema_count):
    x = np.asarray(x, np.float32)
    wpack = _pack_weights(np.asarray(enc_w1), np.asarray(enc_w2), np.asarray(enc_w3),
                          np.asarray(dec_w1), np.asarray(dec_w2), np.asarray(dec_w3),
                          np.asarray(codes))

    key = (STAGES, DEBUG, NOZ2)
    if key not in _CACHE:
        _CACHE[key] = build_nc()
    nc = _CACHE[key]

    in_maps = []
    for core in range(NCORES):
        m = dict(wpack)
        m["im2r"] = _im2row(x[core * P_IMG:(core + 1) * P_IMG])
        in_maps.append(m)

    trace = os.environ.get("KERNEL_PROFILE", "0") == "1"
    if trace:
        trace = _setup_profiling()
    res = run_bass_kernel_spmd(nc, in_maps, list(range(NCORES)), trace=trace)
    _CACHE["res"] = res
    if trace and res.exec_time_ns is not None:
        print(f"HW exec time: {res.exec_time_ns} ns", flush=True)
        _CACHE["exec_time_ns"] = res.exec_time_ns

    B, C, H, W = 32, 64, 63, 63
    x_hat = np.empty((32, 256, 64, 64), np.float32)
    idx_full = np.empty((32, 3969), np.int64)
    z2_sum = 0.0
    smax_sum = 0.0
    for core in range(NCORES):
        r = res.results[core]
        x_hat[core * P_IMG:(core + 1) * P_IMG] = r["xh"].reshape(P_IMG, 256, 64, 64)
        idx_full[core * P_IMG:(core + 1) * P_IMG] = r["idxo"].astype(np.int64)
        z2_sum += float(r["zacc_o"].astype(np.float64).sum())
        smax_sum += float(r["macc_o"].astype(np.float64).sum())

    ema = np.asarray(ema_count, np.float64)
    probs = (ema / ema.sum()).astype(np.float32)
    z_probs = probs[idx_full].reshape(32, 63, 63).astype(np.float32)

    vq_loss = np.float32(0.25 * (z2_sum - smax_sum) / (B * C * H * W))
    return x_hat, z_probs, vq_loss


# revision 12
# speedup vs baseline: 1.1852x; 1.0339x over previous
"""Trainium2 Bass kernel for nn_BottleneckVQa (VQ bottleneck autoencoder).

Data-parallel over 8 NeuronCores: batch 32 -> 4 images/core. Encoder convs run
in f32r (TF32-like, full PE rate), decoder convs in bf16. VQ argmin via
PE transpose + vector max/max_index; codes gathered by indirect DMA.

kernel(**inputs) takes the full unsharded inputs and returns
(x_hat, z_probs, vq_loss) matching the reference.
"""
import os
import sys
import numpy as np

sys.path.insert(0, "/opt/trn_rl_repo")

import ml_dtypes
import concourse.bass as bass
import concourse.tile as tile
from concourse import bacc, mybir
from concourse.bass_utils import run_bass_kernel_spmd
from concourse.masks import make_identity
from contextlib import ExitStack

F32 = mybir.dt.float32
F32R = mybir.dt.float32r
BF16 = mybir.dt.bfloat16
U32 = mybir.dt.uint32
GELU = mybir.ActivationFunctionType.Gelu if os.environ.get("KERNEL_SIM", "0") != "1" else mybir.ActivationFunctionType.Copy
COPY = mybir.ActivationFunctionType.Copy
IDENT = mybir.ActivationFunctionType.Identity

P_IMG = 4            # images per core
NCORES = 8
STAGES = int(os.environ.get("KERNEL_STAGES", "8"))
DEBUG = os.environ.get("KERNEL_DEBUG", "0") == "1"
SIM_SAFE = os.environ.get("KERNEL_SIM", "0") == "1"
NOZ2 = os.environ.get("KERNEL_NOZ2", "0") == "1"
RAW_F32R = False     # f32r dropped: encoder must be exact fp32 (argmin flips)

_CACHE = {}


# ---------------------------------------------------------------- device build
def build_nc():
    nc = bacc.Bacc("TRN2", target_bir_lowering=False, debug=False, num_devices=NCORES)

    dt_c1 = F32
    # inputs
    IM2R = nc.dram_tensor("im2r", [128, P_IMG * 16384], dt_c1, kind="ExternalInput").ap()
    W1P = nc.dram_tensor("w1p", [128, 128], dt_c1, kind="ExternalInput").ap()
    W2P = nc.dram_tensor("w2p", [2, 5, 128, 64], F32, kind="ExternalInput").ap()
    W2P4 = nc.dram_tensor("w2p4", [5, 64, 64], F32, kind="ExternalInput").ap()
    W3P = nc.dram_tensor("w3p", [4, 64, 64], F32, kind="ExternalInput").ap()
    WD1 = nc.dram_tensor("wd1", [2, 128, 512], BF16, kind="ExternalInput").ap()
    WD2 = nc.dram_tensor("wd2", [4, 4, 128, 256], BF16, kind="ExternalInput").ap()
    WD3 = nc.dram_tensor("wd3", [4, 2, 128, 256], BF16, kind="ExternalInput").ap()
    CODES2 = nc.dram_tensor("codes2", [65, 24], F32, kind="ExternalInput").ap()
    CTBL = nc.dram_tensor("ctbl", [24, 128], F32, kind="ExternalInput").ap()

    # outputs
    XH = nc.dram_tensor("xh", [P_IMG, 256, 4096], F32, kind="ExternalOutput").ap()
    IDXO = nc.dram_tensor("idxo", [P_IMG, 3969], U32, kind="ExternalOutput").ap()
    ZACC = nc.dram_tensor("zacc_o", [64, 32], F32, kind="ExternalOutput").ap()
    MACC = nc.dram_tensor("macc_o", [128, 128], F32, kind="ExternalOutput").ap()
    if DEBUG:
        DA1 = nc.dram_tensor("dbg_a1", [128, 66 * 132], F32, kind="ExternalOutput").ap()
        DA2 = nc.dram_tensor("dbg_a2", [64, 4096], F32, kind="ExternalOutput").ap()
        DZ = nc.dram_tensor("dbg_z", [64, 3969], F32, kind="ExternalOutput").ap()
        DS = nc.dram_tensor("dbg_ssb", [24, 3969], F32, kind="ExternalOutput").ap()
        DZQ = nc.dram_tensor("dbg_zqb", [128, 65 * 68], BF16, kind="ExternalOutput").ap()
        DY1 = nc.dram_tensor("dbg_y1", [128, 4096], BF16, kind="ExternalOutput").ap()
        DY2 = nc.dram_tensor("dbg_y2", [128, 65 * 68], BF16, kind="ExternalOutput").ap()

    with tile.TileContext(nc) as tc, ExitStack() as ctx:
        wpool = ctx.enter_context(tc.tile_pool(name="wpool", bufs=1))
        stg = ctx.enter_context(tc.tile_pool(name="stg", bufs=2))
        actp = ctx.enter_context(tc.tile_pool(name="actp", bufs=1))
        im2p = ctx.enter_context(tc.tile_pool(name="im2p", bufs=3))
        vqp = ctx.enter_context(tc.tile_pool(name="vqp", bufs=6))
        outp = ctx.enter_context(tc.tile_pool(name="outp", bufs=3))
        psum = ctx.enter_context(tc.tile_pool(name="psum", bufs=3, space="PSUM"))
        psvq = ctx.enter_context(tc.tile_pool(name="psvq", bufs=3, space="PSUM"))

        # ---- static setup: identities, weights, accumulators ----
        ident = wpool.tile([128, 128], F32, tag="ident")
        make_identity(nc, ident[:])

        w1p = wpool.tile([128, 128], dt_c1, tag="w1p")
        nc.sync.dma_start(w1p[:], W1P)

        w2p = wpool.tile([128, 10 * 64], F32, tag="w2p")
        for g in range(2):
            for kw in range(5):
                nc.sync.dma_start(w2p[:, (g * 5 + kw) * 64:(g * 5 + kw + 1) * 64], W2P[g, kw])
        w2p4 = wpool.tile([64, 5 * 64], F32, tag="w2p4")
        for kw in range(5):
            nc.sync.dma_start(w2p4[:, kw * 64:(kw + 1) * 64], W2P4[kw])
        w3p = wpool.tile([64, 4 * 64], F32, tag="w3p")
        for t_ in range(4):
            nc.sync.dma_start(w3p[:, t_ * 64:(t_ + 1) * 64], W3P[t_])
        codes2 = wpool.tile([65, 24], F32, tag="codes2")
        nc.sync.dma_start(codes2[:], CODES2)

        wd1 = wpool.tile([128, 2 * 512], BF16, tag="wd1")
        for t_ in range(2):
            nc.sync.dma_start(wd1[:, t_ * 512:(t_ + 1) * 512], WD1[t_])
        wd2 = wpool.tile([128, 16 * 256], BF16, tag="wd2")
        for t_ in range(4):
            for s in range(4):
                nc.sync.dma_start(wd2[:, (t_ * 4 + s) * 256:(t_ * 4 + s + 1) * 256], WD2[t_, s])
        wd3 = wpool.tile([128, 8 * 256], BF16, tag="wd3")
        for t_ in range(4):
            for s in range(2):
                nc.sync.dma_start(wd3[:, (t_ * 2 + s) * 256:(t_ * 2 + s + 1) * 256], WD3[t_, s])

        macc = actp.tile([128, 128], F32, tag="macc")
        nc.gpsimd.memset(macc[:], 0.0)
        zacc = actp.tile([64, 32], F32, tag="zacc")
        nc.gpsimd.memset(zacc[:], 0.0)

        # persistent per-image activation buffers (bufs=1 -> reused across images)
        a1 = actp.tile([128, 66 * 132], F32, tag="a1")      # conv1 out, parity-interleaved, padded
        a2 = actp.tile([64, 64 * 64], F32, tag="a2")        # conv2 out, plain
        zt = actp.tile([65, 3969], F32, tag="zt")           # conv3 out (z); row 64 = ones
        zqb_ab = [actp.tile([128, 65 * 68], BF16, name=f"zqb{i}", tag=f"zqb{i}") for i in range(2)]
        y1 = [actp.tile([128, 4096], BF16, name=f"y1_{s}", tag=f"y1_{s}") for s in range(4)]
        y2 = [actp.tile([128, 65 * 68], BF16, name=f"y2_{s}", tag=f"y2_{s}") for s in range(2)]

        a1v = a1[:].rearrange("p (s c) -> p s c", s=66, c=132)
        # zero pad regions once: interiors are fully rewritten every image
        nc.gpsimd.memset(a1[:], 0.0)
        nc.gpsimd.memset(zqb_ab[0][:], 0.0)
        nc.gpsimd.memset(zqb_ab[1][:], 0.0)
        nc.gpsimd.memset(zt[64:65, :], 1.0)
        for s_ in range(2):
            nc.gpsimd.memset(y2[s_][:], 0.0)

        for img in range(P_IMG):
            # ---------------- conv1: im2row matmul, K=75, M=128(dup) ----------------
            for rb in range(32):  # 4 output rows per block
                t = im2p.tile([128, 512], dt_c1, tag="im2t")
                nc.sync.dma_start(t[:], IM2R[:, img * 16384 + rb * 512: img * 16384 + (rb + 1) * 512])
                p = psum.tile([128, 512], F32, tag="mm")
                nc.tensor.matmul(p[:], w1p[:], t[:], start=True, stop=True)
                pv = p[:].rearrange("p (r c) -> p r c", r=4, c=128)
                # rows 4rb+0..3; slot = r//2 + 1; parity r%2 (M duplicated across halves)
                nc.scalar.activation(a1v[0:64, 2 * rb + 1: 2 * rb + 3, 2:130], pv[0:64, 0:4:2, :], GELU)
                nc.scalar.activation(a1v[64:128, 2 * rb + 1: 2 * rb + 3, 2:130], pv[64:128, 1:4:2, :], GELU)

            if DEBUG and img == 0:
                nc.sync.dma_start(DA1, a1[:])
            if STAGES < 2:
                continue
            # ---------------- conv2: K=128 kh-pairs + K=64 tail, M=64 ----------------
            a2v = a2[:].rearrange("p (r c) -> p r c", r=64, c=64)
            for t8 in range(8):
                oh0 = 8 * t8
                p = psum.tile([64, 512], F32, tag="mm")
                first = True
                for g in range(2):
                    for kw in range(5):
                        nc.tensor.matmul(
                            p[:], w2p[:, (g * 5 + kw) * 64:(g * 5 + kw + 1) * 64],
                            a1v[:, oh0 + g: oh0 + g + 8, kw: kw + 128: 2],
                            start=first, stop=False)
                        first = False
                for kw in range(5):
                    nc.tensor.matmul(
                        p[:], w2p4[:, kw * 64:(kw + 1) * 64],
                        a1v[0:64, oh0 + 2: oh0 + 10, kw: kw + 128: 2],
                        start=False, stop=(kw == 4))
                nc.scalar.activation(a2v[:, oh0: oh0 + 8, :], p[:].rearrange("p (r c) -> p r c", r=8, c=64), GELU)

            if DEBUG and img == 0:
                nc.sync.dma_start(DA2, a2[:])
            if STAGES < 3:
                continue
            # ---------------- conv3: K=64, 4 taps, M=64 -> z ----------------
            for t8 in range(8):
                oh0 = 8 * t8
                nr = 8 if t8 < 7 else 7
                p = psum.tile([64, 504], F32, tag="mm")
                for tap in range(4):
                    kh, kw = tap // 2, tap % 2
                    nc.tensor.matmul(
                        p[:, :nr * 63], w3p[:, tap * 64:(tap + 1) * 64],
                        a2v[0:64, oh0 + kh: oh0 + kh + nr, kw: kw + 63],
                        start=(tap == 0), stop=(tap == 3))
                nc.scalar.activation(zt[0:64, oh0 * 63: (oh0 + nr) * 63], p[:, :nr * 63], COPY)
                if not NOZ2:
                    z2s = stg.tile([64, 504], F32, tag="z2s")
                    nc.vector.tensor_mul(z2s[:, :nr * 63], zt[0:64, oh0 * 63: (oh0 + nr) * 63],
                                         zt[0:64, oh0 * 63: (oh0 + nr) * 63])
                    nc.vector.tensor_reduce(
                        out=zacc[:, img * 8 + t8: img * 8 + t8 + 1],
                        in_=z2s[:, :nr * 63], op=mybir.AluOpType.add,
                        axis=mybir.AxisListType.X)

            if DEBUG and img == 0:
                nc.sync.dma_start(DZ, zt[0:64, :])
            if STAGES < 5:
                continue
            # ---------------- VQ: scores-mm -> max/argmax -> gather -> zq ----------------
            zqb = zqb_ab[img % 2]
            zqv = zqb[:].rearrange("p (r c) -> p r c", r=65, c=68)
            for ch in range(32):
                n0 = ch * 126
                n = 126 if ch < 31 else 63
                pT = psvq.tile([126, 24], F32, tag="pT")
                nc.tensor.matmul(pT[:n, :], zt[:, n0:n0 + n], codes2[:], start=True, stop=True)
                sT = vqp.tile([126, 24], F32, tag="sT", bufs=6)
                nc.vector.tensor_copy(sT[:n, :], pT[:n, :])
                mxt = vqp.tile([126, 8], F32, tag="mxt", bufs=6)
                nc.vector.max(mxt[:n, :], sT[:n, :])
                mi = vqp.tile([126, 8], U32, tag="mi", bufs=24)
                nc.vector.max_index(mi[:n, :], mxt[:n, :], sT[:n, :])
                nc.vector.tensor_copy(macc[0:n, img * 32 + ch: img * 32 + ch + 1], mxt[0:n, 0:1])
                nc.sync.dma_start(IDXO[img, n0:n0 + n], mi[0:n, 0:1])
                zqT = vqp.tile([126, 128], F32, tag="zqT", bufs=8)
                nc.gpsimd.indirect_dma_start(
                    out=zqT[:n, :], out_offset=None, in_=CTBL,
                    in_offset=bass.IndirectOffsetOnAxis(ap=mi[0:n, 0:1], axis=0))
                pq = psvq.tile([128, 126], F32, tag="pq", bufs=2)
                nc.tensor.transpose(pq[:, :n], zqT[:n, :], ident[0:n, 0:n])
                # rows 2ch, 2ch+1 -> padded rows 2ch+1, 2ch+2; hi half shifted left 1 col
                nc.scalar.activation(
                    zqv[0:64, 2 * ch + 1: 2 * ch + 1 + n // 63, 1:64],
                    pq[0:64, :n].rearrange("p (r c) -> p r c", r=n // 63, c=63), COPY)
                nc.scalar.activation(
                    zqv[64:128, 2 * ch + 1: 2 * ch + 1 + n // 63, 0:63],
                    pq[64:128, :n].rearrange("p (r c) -> p r c", r=n // 63, c=63), COPY)

            if DEBUG and img == 0:
                nc.sync.dma_start(DZQ, zqb[:])
            if STAGES < 6:
                continue
            # ---------------- dec1: K=64, 4 taps, M=512 (4 chunks) ----------------
            for t8 in range(8):
                oh0 = 8 * t8
                for mch in range(4):
                    p = psum.tile([128, 512], F32, tag="mm")
                    for kh in range(2):
                        nc.tensor.matmul(
                            p[:], wd1[:, kh * 512 + mch * 128: kh * 512 + (mch + 1) * 128],
                            zqv[:, oh0 + kh: oh0 + kh + 8, 0:64],
                            start=(kh == 0), stop=(kh == 1))
                    nc.scalar.activation(y1[mch][:, oh0 * 64: (oh0 + 8) * 64], p[:], GELU)

            if DEBUG and img == 0:
                nc.sync.dma_start(DY1, y1[0][:])
            if STAGES < 7:
                continue
            # ---------------- dec2: K=512 (4 slabs), 4 taps, M=256 (2 chunks) ----------------
            y1v = [y1[s][:].rearrange("p (r c) -> p r c", r=64, c=64) for s in range(4)]
            y2v = [y2[s][:].rearrange("p (r c) -> p r c", r=65, c=68) for s in range(2)]
            for t8 in range(8):
                oh0 = 8 * t8
                nr = 8 if t8 < 7 else 7
                for mch in range(2):
                    p = psum.tile([128, 504], F32, tag="mm")
                    first = True
                    for tap in range(4):
                        kh, kw = tap // 2, tap % 2
                        for s in range(4):
                            nc.tensor.matmul(
                                p[:, :nr * 63],
                                wd2[:, (tap * 4 + s) * 256 + mch * 128: (tap * 4 + s) * 256 + (mch + 1) * 128],
                                y1v[s][:, oh0 + kh: oh0 + kh + nr, kw: kw + 63],
                                start=first, stop=(tap == 3 and s == 3))
                            first = False
                    nc.scalar.activation(
                        y2v[mch][:, oh0 + 1: oh0 + 1 + nr, 1:64],
                        p[:, :nr * 63].rearrange("p (r c) -> p r c", r=nr, c=63), GELU)

            if DEBUG and img == 0:
                nc.sync.dma_start(DY2, y2[0][:])
            if STAGES < 8:
                continue
            # ---------------- dec3: K=256 (2 slabs), 4 taps, M=256 (2 chunks) ----------------
            for t8 in range(8):
                oh0 = 8 * t8
                for mch in range(2):
                    p = psum.tile([128, 512], F32, tag="mm")
                    first = True
                    for tap in range(4):
                        kh, kw = tap // 2, tap % 2
                        for s in range(2):
                            nc.tensor.matmul(
                                p[:],
                                wd3[:, (tap * 2 + s) * 256 + mch * 128: (tap * 2 + s) * 256 + (mch + 1) * 128],
                                y2v[s][:, oh0 + kh: oh0 + kh + 8, kw: kw + 64],
                                start=first, stop=(tap == 3 and s == 1))
                            first = False
                    xo = outp.tile([128, 512], F32, tag="xo")
                    nc.scalar.activation(xo[:], p[:], COPY)
                    nc.sync.dma_start(XH[img, mch * 128:(mch + 1) * 128, oh0 * 64:(oh0 + 8) * 64], xo[:])

        nc.sync.dma_start(MACC, macc[:])
        nc.sync.dma_start(ZACC, zacc[:])

    nc.compile()
    return nc

# memset y2 pads once per image is needed: y2 pad cols/rows written never read?
# dec3 reads padded region -> must be zero. y2 buffers are reused across images;
# interior is fully overwritten each image, pads stay zero from a single memset.


def _setup_profiling():
    """Shim antenv.axon_hooks (absent on this image) with the boot module's
    ctypes NTFF hook, and neuter the artifact upload."""
    try:
        import types
        import concourse.bass_utils as bu
        from trn_agent_boot.trn_boot import _ntff_profile_via_ctypes
        import antenv
        if "antenv.axon_hooks" not in sys.modules:
            hook = _ntff_profile_via_ctypes("/opt/axon/libaxon_pjrt.so")
            if hook is None:
                return False
            mod = types.ModuleType("antenv.axon_hooks")
            mod.get_axon_ntff_profile_hook = lambda: hook
            mod.set_axon_ntff_profile_hook = lambda h: None
            sys.modules["antenv.axon_hooks"] = mod
            antenv.axon_hooks = mod
        bu.upload_artifacts = lambda tmpdir: "local://" + str(tmpdir)
        return True
    except Exception as e:  # pragma: no cover
        print(f"profiling setup failed: {e}", flush=True)
        return False


# ---------------------------------------------------------------- host packing
def _pack_weights(enc_w1, enc_w2, enc_w3, dec_w1, dec_w2, dec_w3, codes):
    bf = ml_dtypes.bfloat16
    w1im = enc_w1.transpose(2, 3, 1, 0).reshape(75, 64).astype(np.float32)
    w1p = np.zeros((128, 128), np.float32)
    w1p[:75, 0:64] = w1im
    w1p[:75, 64:128] = w1im

    w2p = np.zeros((2, 5, 128, 64), np.float32)
    for g in range(2):
        for kw in range(5):
            w2p[g, kw, 0:64] = enc_w2[:, :, 2 * g, kw].T
            w2p[g, kw, 64:128] = enc_w2[:, :, 2 * g + 1, kw].T
    w2p4 = np.stack([enc_w2[:, :, 4, kw].T for kw in range(5)], 0).astype(np.float32)
    w3p = np.stack([enc_w3[:, :, t // 2, t % 2].T for t in range(4)], 0).astype(np.float32)

    # dec1 kw-pair: lhsT[kh] rows 0:64 = w(kh,kw=0), rows 64:128 = w(kh,kw=1)
    wd1 = np.stack(
        [np.concatenate([dec_w1[:, :, kh, 0].T, dec_w1[:, :, kh, 1].T], axis=0)
         for kh in range(2)], 0).astype(bf)
    wd2 = np.stack(
        [np.stack([dec_w2[:, 128 * s:128 * (s + 1), t // 2, t % 2].T for s in range(4)], 0)
         for t in range(4)], 0).astype(bf)
    wd3 = np.stack(
        [np.stack([dec_w3[:, 128 * s:128 * (s + 1), t // 2, t % 2].T for s in range(2)], 0)
         for t in range(4)], 0).astype(bf)

    codes2 = np.concatenate(
        [(2.0 * codes.T), -(codes.astype(np.float64) ** 2).sum(1)[None, :]],
        axis=0).astype(np.float32)                             # [65, 24]
    ctbl = np.concatenate([codes, codes], axis=1).astype(np.float32)   # [24, 128] dup
    return dict(w1p=w1p, w2p=w2p, w2p4=w2p4, w3p=w3p, wd1=wd1, wd2=wd2, wd3=wd3,
                codes2=codes2, ctbl=ctbl)


def _im2row(x4):
    """x4: [4, 3, 256, 256] fp32 -> [75, 4*16384] fp32 (conv1 im2row, pad 2 stride 2)."""
    xp = np.pad(x4, ((0, 0), (0, 0), (2, 2), (2, 2)))
    sl = np.empty((25, 4, 3, 128, 128), np.float32)
    for kh in range(5):
        for kw in range(5):
            sl[kh * 5 + kw] = xp[:, :, kh:kh + 256:2, kw:kw + 256:2]
    out = np.zeros((128, 4 * 16384), np.float32)
    out[:75] = sl.transpose(0, 2, 1, 3, 4).reshape(75, 4 * 16384)
    return out


# ---------------------------------------------------------------- entry point
def kernel(x, enc_w1, enc_w2, enc_w3, dec_w1, dec_w2, dec_w3, codes, ema_count):
    x = np.asarray(x, np.float32)
    wpack = _pack_weights(np.asarray(enc_w1), np.asarray(enc_w2), np.asarray(enc_w3),
                          np.asarray(dec_w1), np.asarray(dec_w2), np.asarray(dec_w3),
                          np.asarray(codes))

    key = (STAGES, DEBUG, NOZ2)
    if key not in _CACHE:
        _CACHE[key] = build_nc()
    nc = _CACHE[key]

    in_maps = []
    for core in range(NCORES):
        m = dict(wpack)
        m["im2r"] = _im2row(x[core * P_IMG:(core + 1) * P_IMG])
        in_maps.append(m)

    trace = os.environ.get("KERNEL_PROFILE", "0") == "1"
    if trace:
        trace = _setup_profiling()
    res = run_bass_kernel_spmd(nc, in_maps, list(range(NCORES)), trace=trace)
    _CACHE["res"] = res
    if trace and res.exec_time_ns is not None:
        print(f"HW exec time: {res.exec_time_ns} ns", flush=True)
        _CACHE["exec_time_ns"] = res.exec_time_ns

    B, C, H, W = 32, 64, 63, 63
    x_hat = np.empty((32, 256, 64, 64), np.float32)
    idx_full = np.empty((32, 3969), np.int64)
    z2_sum = 0.0
    smax_sum = 0.0
    for core in range(NCORES):
        r = res.results[core]
        x_hat[core * P_IMG:(core + 1) * P_IMG] = r["xh"].reshape(P_IMG, 256, 64, 64)
        idx_full[core * P_IMG:(core + 1) * P_IMG] = r["idxo"].astype(np.int64)
        z2_sum += float(r["zacc_o"].astype(np.float64).sum())
        smax_sum += float(r["macc_o"].astype(np.float64).sum())

    ema = np.asarray(ema_count, np.float64)
    probs = (ema / ema.sum()).astype(np.float32)
    z_probs = probs[idx_full].reshape(32, 63, 63).astype(np.float32)

    vq_loss = np.float32(0.25 * (z2_sum - smax_sum) / (B * C * H * W))
    return x_hat, z_probs, vq_loss


# revision 19
# speedup vs baseline: 1.5463x; 1.3046x over previous
"""Trainium2 Bass kernel for nn_BottleneckVQa (VQ bottleneck autoencoder).

Pure data parallel over 8 NeuronCores: batch 32 -> 4 images/core; weights and
the 24x64 codebook replicated. Encoder convs + VQ scores run in exact fp32
(bf16/f32r flip the argmin near ties -> large output error); the decoder runs
in bf16. Conv structure: conv1 via host-side im2row (K=75 padded to 128, M
duplicated so both row parities come straight out of PSUM); conv2 fuses kh
pairs into K=128 against a parity-interleaved layout and runs pairs of row
tiles concurrently in separate PE column groups (tile_position); conv3 is
column-tiled the same way; dec1 fuses the two kw taps into K=128 against a
channel-duplicated, column-shifted zq buffer; dec2/dec3 are dense K=128 bf16.
VQ: scores come out position-major from a [z;1]-as-lhsT matmul, argmax via
vector max/max_index, codes gathered by indirect DMA and transposed back on
the PE. vq_loss partials (sum z^2, sum max-score) and argmin indices are
returned per core; the host assembles vq_loss and z_probs = probs[idx].

kernel(**inputs) takes the full unsharded inputs and returns
(x_hat, z_probs, vq_loss) matching the reference.
"""
import os
import sys
import numpy as np

sys.path.insert(0, "/opt/trn_rl_repo")

import ml_dtypes
import concourse.bass as bass
import concourse.tile as tile
from concourse import bacc, mybir
from concourse.bass_utils import run_bass_kernel_spmd
from concourse.masks import make_identity
from contextlib import ExitStack

F32 = mybir.dt.float32
F32R = mybir.dt.float32r
BF16 = mybir.dt.bfloat16
U32 = mybir.dt.uint32
GELU = mybir.ActivationFunctionType.Gelu if os.environ.get("KERNEL_SIM", "0") != "1" else mybir.ActivationFunctionType.Copy
COPY = mybir.ActivationFunctionType.Copy
IDENT = mybir.ActivationFunctionType.Identity

P_IMG = 4            # images per core
NCORES = 8
STAGES = int(os.environ.get("KERNEL_STAGES", "8"))
DEBUG = os.environ.get("KERNEL_DEBUG", "0") == "1"
SIM_SAFE = os.environ.get("KERNEL_SIM", "0") == "1"
NOZ2 = os.environ.get("KERNEL_NOZ2", "0") == "1"
RAW_F32R = False     # f32r dropped: encoder must be exact fp32 (argmin flips)

_CACHE = {}


# ---------------------------------------------------------------- device build
def build_nc():
    nc = bacc.Bacc("TRN2", target_bir_lowering=False, debug=False, num_devices=NCORES)

    dt_c1 = F32
    # inputs
    IM2R = nc.dram_tensor("im2r", [2, 128, P_IMG * 16384], BF16, kind="ExternalInput").ap()
    W1P = nc.dram_tensor("w1p", [2, 128, 128], BF16, kind="ExternalInput").ap()
    W2P = nc.dram_tensor("w2p", [2, 5, 128, 64], F32, kind="ExternalInput").ap()
    W2P4 = nc.dram_tensor("w2p4", [5, 64, 64], F32, kind="ExternalInput").ap()
    W3P = nc.dram_tensor("w3p", [4, 64, 64], F32, kind="ExternalInput").ap()
    WD1 = nc.dram_tensor("wd1", [2, 128, 512], BF16, kind="ExternalInput").ap()
    WD2 = nc.dram_tensor("wd2", [4, 4, 128, 256], BF16, kind="ExternalInput").ap()
    WD3 = nc.dram_tensor("wd3", [4, 2, 128, 256], BF16, kind="ExternalInput").ap()
    CODES2 = nc.dram_tensor("codes2", [65, 24], F32, kind="ExternalInput").ap()
    CTBL = nc.dram_tensor("ctbl", [24, 128], F32, kind="ExternalInput").ap()

    # outputs
    XH = nc.dram_tensor("xh", [P_IMG, 256, 4096], F32, kind="ExternalOutput").ap()
    IDXO = nc.dram_tensor("idxo", [P_IMG, 3969], U32, kind="ExternalOutput").ap()
    ZACC = nc.dram_tensor("zacc_o", [64, 32], F32, kind="ExternalOutput").ap()
    MACC = nc.dram_tensor("macc_o", [128, 128], F32, kind="ExternalOutput").ap()
    if DEBUG:
        DA1 = nc.dram_tensor("dbg_a1", [128, 66 * 132], F32, kind="ExternalOutput").ap()
        DA2 = nc.dram_tensor("dbg_a2", [64, 4096], F32, kind="ExternalOutput").ap()
        DZ = nc.dram_tensor("dbg_z", [64, 3969], F32, kind="ExternalOutput").ap()
        DS = nc.dram_tensor("dbg_ssb", [24, 3969], F32, kind="ExternalOutput").ap()
        DZQ = nc.dram_tensor("dbg_zqb", [128, 65 * 68], BF16, kind="ExternalOutput").ap()
        DY1 = nc.dram_tensor("dbg_y1", [128, 4096], BF16, kind="ExternalOutput").ap()
        DY2 = nc.dram_tensor("dbg_y2", [128, 65 * 68], BF16, kind="ExternalOutput").ap()

    with tile.TileContext(nc) as tc, ExitStack() as ctx:
        wpool = ctx.enter_context(tc.tile_pool(name="wpool", bufs=1))
        stg = ctx.enter_context(tc.tile_pool(name="stg", bufs=2))
        actp = ctx.enter_context(tc.tile_pool(name="actp", bufs=1))
        im2p = ctx.enter_context(tc.tile_pool(name="im2p", bufs=3))
        vqp = ctx.enter_context(tc.tile_pool(name="vqp", bufs=6))
        outp = ctx.enter_context(tc.tile_pool(name="outp", bufs=3))
        psum = ctx.enter_context(tc.tile_pool(name="psum", bufs=3, space="PSUM"))
        psvq = ctx.enter_context(tc.tile_pool(name="psvq", bufs=3, space="PSUM"))

        # ---- static setup: identities, weights, accumulators ----
        ident = wpool.tile([128, 128], F32, tag="ident")
        make_identity(nc, ident[:])

        w1p = wpool.tile([128, 256], BF16, tag="w1p")
        nc.sync.dma_start(w1p[:, 0:128], W1P[0])
        nc.sync.dma_start(w1p[:, 128:256], W1P[1])

        w2p = wpool.tile([128, 10 * 64], F32, tag="w2p")
        for g in range(2):
            for kw in range(5):
                nc.sync.dma_start(w2p[:, (g * 5 + kw) * 64:(g * 5 + kw + 1) * 64], W2P[g, kw])
        w2p4 = wpool.tile([64, 5 * 64], F32, tag="w2p4")
        for kw in range(5):
            nc.sync.dma_start(w2p4[:, kw * 64:(kw + 1) * 64], W2P4[kw])
        w3p = wpool.tile([64, 4 * 64], F32, tag="w3p")
        for t_ in range(4):
            nc.sync.dma_start(w3p[:, t_ * 64:(t_ + 1) * 64], W3P[t_])
        codes2 = wpool.tile([65, 24], F32, tag="codes2")
        nc.sync.dma_start(codes2[:], CODES2)

        wd1 = wpool.tile([128, 2 * 512], BF16, tag="wd1")
        for t_ in range(2):
            nc.sync.dma_start(wd1[:, t_ * 512:(t_ + 1) * 512], WD1[t_])
        wd2 = wpool.tile([128, 16 * 256], BF16, tag="wd2")
        for t_ in range(4):
            for s in range(4):
                nc.sync.dma_start(wd2[:, (t_ * 4 + s) * 256:(t_ * 4 + s + 1) * 256], WD2[t_, s])
        wd3 = wpool.tile([128, 8 * 256], BF16, tag="wd3")
        for t_ in range(4):
            for s in range(2):
                nc.sync.dma_start(wd3[:, (t_ * 2 + s) * 256:(t_ * 2 + s + 1) * 256], WD3[t_, s])

        macc = actp.tile([128, 128], F32, tag="macc")
        nc.gpsimd.memset(macc[:], 0.0)
        zacc = actp.tile([64, 32], F32, tag="zacc")
        nc.gpsimd.memset(zacc[:], 0.0)

        # persistent per-image activation buffers (bufs=1 -> reused across images)
        a1 = actp.tile([128, 66 * 132], F32, tag="a1")      # conv1 out, parity-interleaved, padded
        a2 = actp.tile([64, 64 * 64], F32, tag="a2")        # conv2 out, plain
        zt = actp.tile([65, 3969], F32, tag="zt")           # conv3 out (z); row 64 = ones
        zqb_ab = [actp.tile([128, 65 * 68], BF16, name=f"zqb{i}", tag=f"zqb{i}") for i in range(2)]
        y1 = [actp.tile([128, 4096], BF16, name=f"y1_{s}", tag=f"y1_{s}") for s in range(4)]
        y2 = [actp.tile([128, 65 * 68], BF16, name=f"y2_{s}", tag=f"y2_{s}") for s in range(2)]

        a1v = a1[:].rearrange("p (s c) -> p s c", s=66, c=132)
        # zero pad regions once: interiors are fully rewritten every image
        nc.gpsimd.memset(a1[:], 0.0)
        nc.gpsimd.memset(zqb_ab[0][:], 0.0)
        nc.gpsimd.memset(zqb_ab[1][:], 0.0)
        nc.gpsimd.memset(zt[64:65, :], 1.0)
        for s_ in range(2):
            nc.gpsimd.memset(y2[s_][:], 0.0)

        for img in range(P_IMG):
            # ---------------- conv1: im2row matmul, K=75, M=128(dup) ----------------
            for rb in range(32):  # 4 output rows per block
                t = im2p.tile([128, 512], dt_c1, tag="im2t")
                nc.sync.dma_start(t[:], IM2R[:, img * 16384 + rb * 512: img * 16384 + (rb + 1) * 512])
                p = psum.tile([128, 512], F32, tag="mm")
                nc.tensor.matmul(p[:], w1p[:], t[:], start=True, stop=True)
                pv = p[:].rearrange("p (r c) -> p r c", r=4, c=128)
                # rows 4rb+0..3; slot = r//2 + 1; parity r%2 (M duplicated across halves)
                nc.scalar.activation(a1v[0:64, 2 * rb + 1: 2 * rb + 3, 2:130], pv[0:64, 0:4:2, :], GELU)
                nc.scalar.activation(a1v[64:128, 2 * rb + 1: 2 * rb + 3, 2:130], pv[64:128, 1:4:2, :], GELU)

            if DEBUG and img == 0:
                nc.sync.dma_start(DA1, a1[:])
            if STAGES < 2:
                continue
            # ------- conv2: col-tiled pairs of 8-row tiles (halves in separate col groups) -------
            a2v = a2[:].rearrange("p (r c) -> p r c", r=64, c=64)
            for t4 in range(4):
                oh0 = 16 * t4
                p = psum.tile([128, 512], F32, tag="mm")
                first = [True, True]
                for g in range(2):
                    for kw in range(5):
                        lhs = w2p[:, (g * 5 + kw) * 64:(g * 5 + kw + 1) * 64]
                        for hf in range(2):
                            nc.tensor.matmul(
                                p[64 * hf: 64 * hf + 64, :], lhs,
                                a1v[:, oh0 + 8 * hf + g: oh0 + 8 * hf + g + 8, kw: kw + 128: 2],
                                start=first[hf], stop=False, tile_position=(0, 64 * hf))
                            first[hf] = False
                for kw in range(5):
                    lhs4 = w2p4[:, kw * 64:(kw + 1) * 64]
                    for hf in range(2):
                        nc.tensor.matmul(
                            p[64 * hf: 64 * hf + 64, :], lhs4,
                            a1v[0:64, oh0 + 8 * hf + 2: oh0 + 8 * hf + 10, kw: kw + 128: 2],
                            start=False, stop=(kw == 4), tile_position=(0, 64 * hf))
                nc.scalar.activation(a2v[:, oh0: oh0 + 8, :], p[0:64, :].rearrange("p (r c) -> p r c", r=8, c=64), GELU)
                c2s = stg.tile([128, 512], F32, tag="c2s")
                nc.scalar.activation(c2s[64:128, :], p[64:128, :], GELU)
                nc.sync.dma_start(a2v[:, oh0 + 8: oh0 + 16, :],
                                  c2s[64:128, :].rearrange("p (r c) -> p r c", r=8, c=64))

            if DEBUG and img == 0:
                nc.sync.dma_start(DA2, a2[:])
            if STAGES < 3:
                continue
            # ------- conv3: col-tiled pairs, K=64, 4 taps -> z -------
            for t4 in range(4):
                oh0 = 16 * t4
                nr1 = 8 if t4 < 3 else 7
                p = psum.tile([128, 504], F32, tag="mm")
                for tap in range(4):
                    kh, kw = tap // 2, tap % 2
                    lhs = w3p[:, tap * 64:(tap + 1) * 64]
                    nc.tensor.matmul(
                        p[0:64, :8 * 63], lhs,
                        a2v[0:64, oh0 + kh: oh0 + kh + 8, kw: kw + 63],
                        start=(tap == 0), stop=(tap == 3), tile_position=(0, 0))
                    nc.tensor.matmul(
                        p[64:128, :nr1 * 63], lhs,
                        a2v[0:64, oh0 + 8 + kh: oh0 + 8 + kh + nr1, kw: kw + 63],
                        start=(tap == 0), stop=(tap == 3), tile_position=(0, 64))
                nc.scalar.activation(zt[0:64, oh0 * 63: (oh0 + 8) * 63], p[0:64, :8 * 63], COPY)
                c3s = stg.tile([128, 504], F32, tag="c3s")
                nc.scalar.activation(c3s[64:128, :nr1 * 63], p[64:128, :nr1 * 63], COPY)
                nc.sync.dma_start(zt[0:64, (oh0 + 8) * 63: (oh0 + 8 + nr1) * 63],
                                  c3s[64:128, :nr1 * 63])
                if not NOZ2:
                    for hf, nrh in ((0, 8), (1, nr1)):
                        z2s = stg.tile([64, 504], F32, tag="z2s")
                        nc.vector.tensor_mul(
                            z2s[:, :nrh * 63],
                            zt[0:64, (oh0 + 8 * hf) * 63: (oh0 + 8 * hf + nrh) * 63],
                            zt[0:64, (oh0 + 8 * hf) * 63: (oh0 + 8 * hf + nrh) * 63])
                        nc.vector.tensor_reduce(
                            out=zacc[:, img * 8 + 2 * t4 + hf: img * 8 + 2 * t4 + hf + 1],
                            in_=z2s[:, :nrh * 63], op=mybir.AluOpType.add,
                            axis=mybir.AxisListType.X)

            if DEBUG and img == 0:
                nc.sync.dma_start(DZ, zt[0:64, :])
            if STAGES < 5:
                continue
            # ---------------- VQ: scores-mm -> max/argmax -> gather -> zq ----------------
            zqb = zqb_ab[img % 2]
            zqv = zqb[:].rearrange("p (r c) -> p r c", r=65, c=68)
            for ch in range(32):
                n0 = ch * 126
                n = 126 if ch < 31 else 63
                pT = psvq.tile([126, 24], F32, tag="pT")
                nc.tensor.matmul(pT[:n, :], zt[:, n0:n0 + n], codes2[:], start=True, stop=True)
                sT = vqp.tile([126, 24], F32, tag="sT", bufs=6)
                nc.vector.tensor_copy(sT[:n, :], pT[:n, :])
                mxt = vqp.tile([126, 8], F32, tag="mxt", bufs=6)
                nc.vector.max(mxt[:n, :], sT[:n, :])
                mi = vqp.tile([126, 8], U32, tag="mi", bufs=32)
                nc.vector.max_index(mi[:n, :], mxt[:n, :], sT[:n, :])
                nc.vector.tensor_copy(macc[0:n, img * 32 + ch: img * 32 + ch + 1], mxt[0:n, 0:1])
                nc.sync.dma_start(IDXO[img, n0:n0 + n], mi[0:n, 0:1])
                zqT = vqp.tile([126, 128], F32, tag="zqT", bufs=16)
                nc.gpsimd.indirect_dma_start(
                    out=zqT[:n, :], out_offset=None, in_=CTBL,
                    in_offset=bass.IndirectOffsetOnAxis(ap=mi[0:n, 0:1], axis=0))
                pq = psvq.tile([128, 126], F32, tag="pq", bufs=2)
                nc.tensor.transpose(pq[:, :n], zqT[:n, :], ident[0:n, 0:n])
                # rows 2ch, 2ch+1 -> padded rows 2ch+1, 2ch+2; hi half shifted left 1 col
                nc.scalar.activation(
                    zqv[0:64, 2 * ch + 1: 2 * ch + 1 + n // 63, 1:64],
                    pq[0:64, :n].rearrange("p (r c) -> p r c", r=n // 63, c=63), COPY)
                nc.scalar.activation(
                    zqv[64:128, 2 * ch + 1: 2 * ch + 1 + n // 63, 0:63],
                    pq[64:128, :n].rearrange("p (r c) -> p r c", r=n // 63, c=63), COPY)

            if DEBUG and img == 0:
                nc.sync.dma_start(DZQ, zqb[:])
            if STAGES < 6:
                continue
            # ---------------- dec1: K=64, 4 taps, M=512 (4 chunks) ----------------
            for t8 in range(8):
                oh0 = 8 * t8
                for mch in range(4):
                    p = psum.tile([128, 512], F32, tag="mm")
                    for kh in range(2):
                        nc.tensor.matmul(
                            p[:], wd1[:, kh * 512 + mch * 128: kh * 512 + (mch + 1) * 128],
                            zqv[:, oh0 + kh: oh0 + kh + 8, 0:64],
                            start=(kh == 0), stop=(kh == 1))
                    nc.scalar.activation(y1[mch][:, oh0 * 64: (oh0 + 8) * 64], p[:], GELU)

            if DEBUG and img == 0:
                nc.sync.dma_start(DY1, y1[0][:])
            if STAGES < 7:
                continue
            # ---------------- dec2: K=512 (4 slabs), 4 taps, M=256 (2 chunks) ----------------
            y1v = [y1[s][:].rearrange("p (r c) -> p r c", r=64, c=64) for s in range(4)]
            y2v = [y2[s][:].rearrange("p (r c) -> p r c", r=65, c=68) for s in range(2)]
            for t8 in range(8):
                oh0 = 8 * t8
                nr = 8 if t8 < 7 else 7
                for mch in range(2):
                    p = psum.tile([128, 504], F32, tag="mm")
                    first = True
                    for tap in range(4):
                        kh, kw = tap // 2, tap % 2
                        for s in range(4):
                            nc.tensor.matmul(
                                p[:, :nr * 63],
                                wd2[:, (tap * 4 + s) * 256 + mch * 128: (tap * 4 + s) * 256 + (mch + 1) * 128],
                                y1v[s][:, oh0 + kh: oh0 + kh + nr, kw: kw + 63],
                                start=first, stop=(tap == 3 and s == 3))
                            first = False
                    nc.scalar.activation(
                        y2v[mch][:, oh0 + 1: oh0 + 1 + nr, 1:64],
                        p[:, :nr * 63].rearrange("p (r c) -> p r c", r=nr, c=63), GELU)

            if DEBUG and img == 0:
                nc.sync.dma_start(DY2, y2[0][:])
            if STAGES < 8:
                continue
            # ---------------- dec3: K=256 (2 slabs), 4 taps, M=256 (2 chunks) ----------------
            for t8 in range(8):
                oh0 = 8 * t8
                for mch in range(2):
                    p = psum.tile([128, 512], F32, tag="mm")
                    first = True
                    for tap in range(4):
                        kh, kw = tap // 2, tap % 2
                        for s in range(2):
                            nc.tensor.matmul(
                                p[:],
                                wd3[:, (tap * 2 + s) * 256 + mch * 128: (tap * 2 + s) * 256 + (mch + 1) * 128],
                                y2v[s][:, oh0 + kh: oh0 + kh + 8, kw: kw + 64],
                                start=first, stop=(tap == 3 and s == 1))
                            first = False
                    xo = outp.tile([128, 512], F32, tag="xo")
                    nc.scalar.activation(xo[:], p[:], COPY)
                    nc.sync.dma_start(XH[img, mch * 128:(mch + 1) * 128, oh0 * 64:(oh0 + 8) * 64], xo[:])

        nc.sync.dma_start(MACC, macc[:])
        nc.sync.dma_start(ZACC, zacc[:])

    nc.compile()
    return nc

# memset y2 pads once per image is needed: y2 pad cols/rows written never read?
# dec3 reads padded region -> must be zero. y2 buffers are reused across images;
# interior is fully overwritten each image, pads stay zero from a single memset.


def _setup_profiling():
    """Shim antenv.axon_hooks (absent on this image) with the boot module's
    ctypes NTFF hook, and neuter the artifact upload."""
    try:
        import types
        import concourse.bass_utils as bu
        from trn_agent_boot.trn_boot import _ntff_profile_via_ctypes
        import antenv
        if "antenv.axon_hooks" not in sys.modules:
            hook = _ntff_profile_via_ctypes("/opt/axon/libaxon_pjrt.so")
            if hook is None:
                return False
            mod = types.ModuleType("antenv.axon_hooks")
            mod.get_axon_ntff_profile_hook = lambda: hook
            mod.set_axon_ntff_profile_hook = lambda h: None
            sys.modules["antenv.axon_hooks"] = mod
            antenv.axon_hooks = mod
        bu.upload_artifacts = lambda tmpdir: "local://" + str(tmpdir)
        return True
    except Exception as e:  # pragma: no cover
        print(f"profiling setup failed: {e}", flush=True)
        return False


# ---------------------------------------------------------------- host packing
def _pack_weights(enc_w1, enc_w2, enc_w3, dec_w1, dec_w2, dec_w3, codes):
    bf = ml_dtypes.bfloat16
    w1im = enc_w1.transpose(2, 3, 1, 0).reshape(75, 64).astype(np.float32)
    w1f = np.zeros((128, 128), np.float32)
    w1f[:75, 0:64] = w1im
    w1f[:75, 64:128] = w1im
    w1h = w1f.astype(bf)
    w1l = (w1f - w1h.astype(np.float32)).astype(bf)
    w1p = np.stack([w1h, w1l], 0)

    w2p = np.zeros((2, 5, 128, 64), np.float32)
    for g in range(2):
        for kw in range(5):
            w2p[g, kw, 0:64] = enc_w2[:, :, 2 * g, kw].T
            w2p[g, kw, 64:128] = enc_w2[:, :, 2 * g + 1, kw].T
    w2p4 = np.stack([enc_w2[:, :, 4, kw].T for kw in range(5)], 0).astype(np.float32)
    w3p = np.stack([enc_w3[:, :, t // 2, t % 2].T for t in range(4)], 0).astype(np.float32)

    # dec1 kw-pair: lhsT[kh] rows 0:64 = w(kh,kw=0), rows 64:128 = w(kh,kw=1)
    wd1 = np.stack(
        [np.concatenate([dec_w1[:, :, kh, 0].T, dec_w1[:, :, kh, 1].T], axis=0)
         for kh in range(2)], 0).astype(bf)
    wd2 = np.stack(
        [np.stack([dec_w2[:, 128 * s:128 * (s + 1), t // 2, t % 2].T for s in range(4)], 0)
         for t in range(4)], 0).astype(bf)
    wd3 = np.stack(
        [np.stack([dec_w3[:, 128 * s:128 * (s + 1), t // 2, t % 2].T for s in range(2)], 0)
         for t in range(4)], 0).astype(bf)

    codes2 = np.concatenate(
        [(2.0 * codes.T), -(codes.astype(np.float64) ** 2).sum(1)[None, :]],
        axis=0).astype(np.float32)                             # [65, 24]
    ctbl = np.concatenate([codes, codes], axis=1).astype(np.float32)   # [24, 128] dup
    return dict(w1p=w1p, w2p=w2p, w2p4=w2p4, w3p=w3p, wd1=wd1, wd2=wd2, wd3=wd3,
                codes2=codes2, ctbl=ctbl)


def _im2row(x4):
    """x4: [4, 3, 256, 256] fp32 -> [75, 4*16384] fp32 (conv1 im2row, pad 2 stride 2)."""
    xp = np.pad(x4, ((0, 0), (0, 0), (2, 2), (2, 2)))
    sl = np.empty((25, 4, 3, 128, 128), np.float32)
    for kh in range(5):
        for kw in range(5):
            sl[kh * 5 + kw] = xp[:, :, kh:kh + 256:2, kw:kw + 256:2]
    import ml_dtypes as _md
    out = np.zeros((128, 4 * 16384), np.float32)
    out[:75] = sl.transpose(0, 2, 1, 3, 4).reshape(75, 4 * 16384)
    hi = out.astype(_md.bfloat16)
    lo = (out - hi.astype(np.float32)).astype(_md.bfloat16)
    return np.stack([hi, lo], 0)


# ---------------------------------------------------------------- entry point
def kernel(x, enc_w1, enc_w2, enc_w3, dec_w1, dec_w2, dec_w3, codes, ema_count):
    x = np.asarray(x, np.float32)
    wpack = _pack_weights(np.asarray(enc_w1), np.asarray(enc_w2), np.asarray(enc_w3),
                          np.asarray(dec_w1), np.asarray(dec_w2), np.asarray(dec_w3),
                          np.asarray(codes))

    key = (STAGES, DEBUG, NOZ2)
    if key not in _CACHE:
        _CACHE[key] = build_nc()
    nc = _CACHE[key]

    in_maps = []
    for core in range(NCORES):
        m = dict(wpack)
        m["im2r"] = _im2row(x[core * P_IMG:(core + 1) * P_IMG])
        in_maps.append(m)

    trace = os.environ.get("KERNEL_PROFILE", "0") == "1"
    if trace:
        trace = _setup_profiling()
    res = run_bass_kernel_spmd(nc, in_maps, list(range(NCORES)), trace=trace)
    _CACHE["res"] = res
    if trace and res.exec_time_ns is not None:
        print(f"HW exec time: {res.exec_time_ns} ns", flush=True)
        _CACHE["exec_time_ns"] = res.exec_time_ns

    B, C, H, W = 32, 64, 63, 63
    x_hat = np.empty((32, 256, 64, 64), np.float32)
    idx_full = np.empty((32, 3969), np.int64)
    z2_sum = 0.0
    smax_sum = 0.0
    for core in range(NCORES):
        r = res.results[core]
        x_hat[core * P_IMG:(core + 1) * P_IMG] = r["xh"].reshape(P_IMG, 256, 64, 64)
        idx_full[core * P_IMG:(core + 1) * P_IMG] = r["idxo"].astype(np.int64)
        z2_sum += float(r["zacc_o"].astype(np.float64).sum())
        smax_sum += float(r["macc_o"].astype(np.float64).sum())

    ema = np.asarray(ema_count, np.float64)
    probs = (ema / ema.sum()).astype(np.float32)
    z_probs = probs[idx_full].reshape(32, 63, 63).astype(np.float32)

    vq_loss = np.float32(0.25 * (z2_sum - smax_sum) / (B * C * H * W))
    return x_hat, z_probs, vq_loss


# revision 20
# speedup vs baseline: 1.6530x; 1.0690x over previous
"""Trainium2 Bass kernel for nn_BottleneckVQa (VQ bottleneck autoencoder).

Pure data parallel over 8 NeuronCores: batch 32 -> 4 images/core; weights and
the 24x64 codebook replicated. Encoder convs + VQ scores run in exact fp32
(bf16/f32r flip the argmin near ties -> large output error); the decoder runs
in bf16. Conv structure: conv1 via host-side im2row (K=75 padded to 128, M
duplicated so both row parities come straight out of PSUM); conv2 fuses kh
pairs into K=128 against a parity-interleaved layout and runs pairs of row
tiles concurrently in separate PE column groups (tile_position); conv3 is
column-tiled the same way; dec1 fuses the two kw taps into K=128 against a
channel-duplicated, column-shifted zq buffer; dec2/dec3 are dense K=128 bf16.
VQ: scores come out position-major from a [z;1]-as-lhsT matmul, argmax via
vector max/max_index, codes gathered by indirect DMA and transposed back on
the PE. vq_loss partials (sum z^2, sum max-score) and argmin indices are
returned per core; the host assembles vq_loss and z_probs = probs[idx].

kernel(**inputs) takes the full unsharded inputs and returns
(x_hat, z_probs, vq_loss) matching the reference.
"""
import os
import sys
import numpy as np

sys.path.insert(0, "/opt/trn_rl_repo")

import ml_dtypes
import concourse.bass as bass
import concourse.tile as tile
from concourse import bacc, mybir
from concourse.bass_utils import run_bass_kernel_spmd
from concourse.masks import make_identity
from contextlib import ExitStack

F32 = mybir.dt.float32
F32R = mybir.dt.float32r
BF16 = mybir.dt.bfloat16
U32 = mybir.dt.uint32
GELU = mybir.ActivationFunctionType.Gelu if os.environ.get("KERNEL_SIM", "0") != "1" else mybir.ActivationFunctionType.Copy
COPY = mybir.ActivationFunctionType.Copy
IDENT = mybir.ActivationFunctionType.Identity

P_IMG = 4            # images per core
NCORES = 8
STAGES = int(os.environ.get("KERNEL_STAGES", "8"))
DEBUG = os.environ.get("KERNEL_DEBUG", "0") == "1"
SIM_SAFE = os.environ.get("KERNEL_SIM", "0") == "1"
NOZ2 = os.environ.get("KERNEL_NOZ2", "0") == "1"
RAW_F32R = False     # f32r dropped: encoder must be exact fp32 (argmin flips)

_CACHE = {}


# ---------------------------------------------------------------- device build
def build_nc():
    nc = bacc.Bacc("TRN2", target_bir_lowering=False, debug=False, num_devices=NCORES)

    dt_c1 = F32
    # inputs
    IM2R = nc.dram_tensor("im2r", [2, 128, P_IMG * 16384], BF16, kind="ExternalInput").ap()
    W1P = nc.dram_tensor("w1p", [2, 128, 128], BF16, kind="ExternalInput").ap()
    W2P = nc.dram_tensor("w2p", [2, 5, 128, 64], F32, kind="ExternalInput").ap()
    W2P4 = nc.dram_tensor("w2p4", [5, 64, 64], F32, kind="ExternalInput").ap()
    W3P = nc.dram_tensor("w3p", [4, 64, 64], F32, kind="ExternalInput").ap()
    WD1 = nc.dram_tensor("wd1", [2, 128, 512], BF16, kind="ExternalInput").ap()
    WD2 = nc.dram_tensor("wd2", [4, 4, 128, 256], BF16, kind="ExternalInput").ap()
    WD3 = nc.dram_tensor("wd3", [4, 2, 128, 256], BF16, kind="ExternalInput").ap()
    CODES2 = nc.dram_tensor("codes2", [65, 24], F32, kind="ExternalInput").ap()
    CTBL = nc.dram_tensor("ctbl", [24, 128], F32, kind="ExternalInput").ap()

    # outputs
    XH = nc.dram_tensor("xh", [P_IMG, 256, 4096], F32, kind="ExternalOutput").ap()
    IDXO = nc.dram_tensor("idxo", [P_IMG, 3969], U32, kind="ExternalOutput").ap()
    ZACC = nc.dram_tensor("zacc_o", [64, 32], F32, kind="ExternalOutput").ap()
    MACC = nc.dram_tensor("macc_o", [128, 128], F32, kind="ExternalOutput").ap()
    if DEBUG:
        DA1 = nc.dram_tensor("dbg_a1", [128, 66 * 132], F32, kind="ExternalOutput").ap()
        DA2 = nc.dram_tensor("dbg_a2", [64, 4096], F32, kind="ExternalOutput").ap()
        DZ = nc.dram_tensor("dbg_z", [64, 3969], F32, kind="ExternalOutput").ap()
        DS = nc.dram_tensor("dbg_ssb", [24, 3969], F32, kind="ExternalOutput").ap()
        DZQ = nc.dram_tensor("dbg_zqb", [128, 65 * 68], BF16, kind="ExternalOutput").ap()
        DY1 = nc.dram_tensor("dbg_y1", [128, 4096], BF16, kind="ExternalOutput").ap()
        DY2 = nc.dram_tensor("dbg_y2", [128, 65 * 68], BF16, kind="ExternalOutput").ap()

    with tile.TileContext(nc) as tc, ExitStack() as ctx:
        wpool = ctx.enter_context(tc.tile_pool(name="wpool", bufs=1))
        stg = ctx.enter_context(tc.tile_pool(name="stg", bufs=2))
        actp = ctx.enter_context(tc.tile_pool(name="actp", bufs=1))
        im2p = ctx.enter_context(tc.tile_pool(name="im2p", bufs=6))
        vqp = ctx.enter_context(tc.tile_pool(name="vqp", bufs=6))
        outp = ctx.enter_context(tc.tile_pool(name="outp", bufs=3))
        psum = ctx.enter_context(tc.tile_pool(name="psum", bufs=3, space="PSUM"))
        psvq = ctx.enter_context(tc.tile_pool(name="psvq", bufs=3, space="PSUM"))

        # ---- static setup: identities, weights, accumulators ----
        ident = wpool.tile([128, 128], F32, tag="ident")
        make_identity(nc, ident[:])

        w1p = wpool.tile([128, 256], BF16, tag="w1p")
        nc.sync.dma_start(w1p[:, 0:128], W1P[0])
        nc.sync.dma_start(w1p[:, 128:256], W1P[1])

        w2p = wpool.tile([128, 10 * 64], F32, tag="w2p")
        for g in range(2):
            for kw in range(5):
                nc.sync.dma_start(w2p[:, (g * 5 + kw) * 64:(g * 5 + kw + 1) * 64], W2P[g, kw])
        w2p4 = wpool.tile([64, 5 * 64], F32, tag="w2p4")
        for kw in range(5):
            nc.sync.dma_start(w2p4[:, kw * 64:(kw + 1) * 64], W2P4[kw])
        w3p = wpool.tile([64, 4 * 64], F32, tag="w3p")
        for t_ in range(4):
            nc.sync.dma_start(w3p[:, t_ * 64:(t_ + 1) * 64], W3P[t_])
        codes2 = wpool.tile([65, 24], F32, tag="codes2")
        nc.sync.dma_start(codes2[:], CODES2)

        wd1 = wpool.tile([128, 2 * 512], BF16, tag="wd1")
        for t_ in range(2):
            nc.sync.dma_start(wd1[:, t_ * 512:(t_ + 1) * 512], WD1[t_])
        wd2 = wpool.tile([128, 16 * 256], BF16, tag="wd2")
        for t_ in range(4):
            for s in range(4):
                nc.sync.dma_start(wd2[:, (t_ * 4 + s) * 256:(t_ * 4 + s + 1) * 256], WD2[t_, s])
        wd3 = wpool.tile([128, 8 * 256], BF16, tag="wd3")
        for t_ in range(4):
            for s in range(2):
                nc.sync.dma_start(wd3[:, (t_ * 2 + s) * 256:(t_ * 2 + s + 1) * 256], WD3[t_, s])

        macc = actp.tile([128, 128], F32, tag="macc")
        nc.gpsimd.memset(macc[:], 0.0)
        zacc = actp.tile([64, 32], F32, tag="zacc")
        nc.gpsimd.memset(zacc[:], 0.0)

        # persistent per-image activation buffers (bufs=1 -> reused across images)
        a1 = actp.tile([128, 66 * 132], F32, tag="a1")      # conv1 out, parity-interleaved, padded
        a2 = actp.tile([64, 64 * 64], F32, tag="a2")        # conv2 out, plain
        zt = actp.tile([65, 3969], F32, tag="zt")           # conv3 out (z); row 64 = ones
        zqb_ab = [actp.tile([128, 65 * 68], BF16, name=f"zqb{i}", tag=f"zqb{i}") for i in range(2)]
        y1 = [actp.tile([128, 4096], BF16, name=f"y1_{s}", tag=f"y1_{s}") for s in range(4)]
        y2 = [actp.tile([128, 65 * 68], BF16, name=f"y2_{s}", tag=f"y2_{s}") for s in range(2)]

        a1v = a1[:].rearrange("p (s c) -> p s c", s=66, c=132)
        # zero pad regions once: interiors are fully rewritten every image
        nc.gpsimd.memset(a1[:], 0.0)
        nc.gpsimd.memset(zqb_ab[0][:], 0.0)
        nc.gpsimd.memset(zqb_ab[1][:], 0.0)
        nc.gpsimd.memset(zt[64:65, :], 1.0)
        for s_ in range(2):
            nc.gpsimd.memset(y2[s_][:], 0.0)

        for img in range(P_IMG):
            # ---------------- conv1: im2row matmul, K=75, M=128(dup) ----------------
            for rb in range(32):  # 4 output rows per block
                t = im2p.tile([128, 512], dt_c1, tag="im2t")
                nc.sync.dma_start(t[:], IM2R[:, img * 16384 + rb * 512: img * 16384 + (rb + 1) * 512])
                p = psum.tile([128, 512], F32, tag="mm")
                nc.tensor.matmul(p[:], w1p[:], t[:], start=True, stop=True)
                pv = p[:].rearrange("p (r c) -> p r c", r=4, c=128)
                # rows 4rb+0..3; slot = r//2 + 1; parity r%2 (M duplicated across halves)
                nc.scalar.activation(a1v[0:64, 2 * rb + 1: 2 * rb + 3, 2:130], pv[0:64, 0:4:2, :], GELU)
                nc.scalar.activation(a1v[64:128, 2 * rb + 1: 2 * rb + 3, 2:130], pv[64:128, 1:4:2, :], GELU)

            if DEBUG and img == 0:
                nc.sync.dma_start(DA1, a1[:])
            if STAGES < 2:
                continue
            # ------- conv2: col-tiled pairs of 8-row tiles (halves in separate col groups) -------
            a2v = a2[:].rearrange("p (r c) -> p r c", r=64, c=64)
            for t4 in range(4):
                oh0 = 16 * t4
                p = psum.tile([128, 512], F32, tag="mm")
                first = [True, True]
                for g in range(2):
                    for kw in range(5):
                        lhs = w2p[:, (g * 5 + kw) * 64:(g * 5 + kw + 1) * 64]
                        for hf in range(2):
                            nc.tensor.matmul(
                                p[64 * hf: 64 * hf + 64, :], lhs,
                                a1v[:, oh0 + 8 * hf + g: oh0 + 8 * hf + g + 8, kw: kw + 128: 2],
                                start=first[hf], stop=False, tile_position=(0, 64 * hf))
                            first[hf] = False
                for kw in range(5):
                    lhs4 = w2p4[:, kw * 64:(kw + 1) * 64]
                    for hf in range(2):
                        nc.tensor.matmul(
                            p[64 * hf: 64 * hf + 64, :], lhs4,
                            a1v[0:64, oh0 + 8 * hf + 2: oh0 + 8 * hf + 10, kw: kw + 128: 2],
                            start=False, stop=(kw == 4), tile_position=(0, 64 * hf))
                nc.scalar.activation(a2v[:, oh0: oh0 + 8, :], p[0:64, :].rearrange("p (r c) -> p r c", r=8, c=64), GELU)
                c2s = stg.tile([128, 512], F32, tag="c2s")
                nc.scalar.activation(c2s[64:128, :], p[64:128, :], GELU)
                nc.sync.dma_start(a2v[:, oh0 + 8: oh0 + 16, :],
                                  c2s[64:128, :].rearrange("p (r c) -> p r c", r=8, c=64))

            if DEBUG and img == 0:
                nc.sync.dma_start(DA2, a2[:])
            if STAGES < 3:
                continue
            # ------- conv3: col-tiled pairs, K=64, 4 taps -> z -------
            for t4 in range(4):
                oh0 = 16 * t4
                nr1 = 8 if t4 < 3 else 7
                p = psum.tile([128, 504], F32, tag="mm")
                for tap in range(4):
                    kh, kw = tap // 2, tap % 2
                    lhs = w3p[:, tap * 64:(tap + 1) * 64]
                    nc.tensor.matmul(
                        p[0:64, :8 * 63], lhs,
                        a2v[0:64, oh0 + kh: oh0 + kh + 8, kw: kw + 63],
                        start=(tap == 0), stop=(tap == 3), tile_position=(0, 0))
                    nc.tensor.matmul(
                        p[64:128, :nr1 * 63], lhs,
                        a2v[0:64, oh0 + 8 + kh: oh0 + 8 + kh + nr1, kw: kw + 63],
                        start=(tap == 0), stop=(tap == 3), tile_position=(0, 64))
                nc.scalar.activation(zt[0:64, oh0 * 63: (oh0 + 8) * 63], p[0:64, :8 * 63], COPY)
                c3s = stg.tile([128, 504], F32, tag="c3s")
                nc.scalar.activation(c3s[64:128, :nr1 * 63], p[64:128, :nr1 * 63], COPY)
                nc.sync.dma_start(zt[0:64, (oh0 + 8) * 63: (oh0 + 8 + nr1) * 63],
                                  c3s[64:128, :nr1 * 63])
                if not NOZ2:
                    for hf, nrh in ((0, 8), (1, nr1)):
                        z2s = stg.tile([64, 504], F32, tag="z2s")
                        nc.vector.tensor_mul(
                            z2s[:, :nrh * 63],
                            zt[0:64, (oh0 + 8 * hf) * 63: (oh0 + 8 * hf + nrh) * 63],
                            zt[0:64, (oh0 + 8 * hf) * 63: (oh0 + 8 * hf + nrh) * 63])
                        nc.vector.tensor_reduce(
                            out=zacc[:, img * 8 + 2 * t4 + hf: img * 8 + 2 * t4 + hf + 1],
                            in_=z2s[:, :nrh * 63], op=mybir.AluOpType.add,
                            axis=mybir.AxisListType.X)

            if DEBUG and img == 0:
                nc.sync.dma_start(DZ, zt[0:64, :])
            if STAGES < 5:
                continue
            # ---------------- VQ: scores-mm -> max/argmax -> gather -> zq ----------------
            zqb = zqb_ab[img % 2]
            zqv = zqb[:].rearrange("p (r c) -> p r c", r=65, c=68)
            for ch in range(32):
                n0 = ch * 126
                n = 126 if ch < 31 else 63
                pT = psvq.tile([126, 24], F32, tag="pT")
                nc.tensor.matmul(pT[:n, :], zt[:, n0:n0 + n], codes2[:], start=True, stop=True)
                sT = vqp.tile([126, 24], F32, tag="sT", bufs=6)
                nc.vector.tensor_copy(sT[:n, :], pT[:n, :])
                mxt = vqp.tile([126, 8], F32, tag="mxt", bufs=6)
                nc.vector.max(mxt[:n, :], sT[:n, :])
                mi = vqp.tile([126, 8], U32, tag="mi", bufs=32)
                nc.vector.max_index(mi[:n, :], mxt[:n, :], sT[:n, :])
                nc.vector.tensor_copy(macc[0:n, img * 32 + ch: img * 32 + ch + 1], mxt[0:n, 0:1])
                nc.sync.dma_start(IDXO[img, n0:n0 + n], mi[0:n, 0:1])
                zqT = vqp.tile([126, 128], F32, tag="zqT", bufs=20)
                nc.gpsimd.indirect_dma_start(
                    out=zqT[:n, :], out_offset=None, in_=CTBL,
                    in_offset=bass.IndirectOffsetOnAxis(ap=mi[0:n, 0:1], axis=0))
                pq = psvq.tile([128, 126], F32, tag="pq", bufs=2)
                nc.tensor.transpose(pq[:, :n], zqT[:n, :], ident[0:n, 0:n])
                # rows 2ch, 2ch+1 -> padded rows 2ch+1, 2ch+2; hi half shifted left 1 col
                nc.scalar.activation(
                    zqv[0:64, 2 * ch + 1: 2 * ch + 1 + n // 63, 1:64],
                    pq[0:64, :n].rearrange("p (r c) -> p r c", r=n // 63, c=63), COPY)
                nc.scalar.activation(
                    zqv[64:128, 2 * ch + 1: 2 * ch + 1 + n // 63, 0:63],
                    pq[64:128, :n].rearrange("p (r c) -> p r c", r=n // 63, c=63), COPY)

            if DEBUG and img == 0:
                nc.sync.dma_start(DZQ, zqb[:])
            if STAGES < 6:
                continue
            # ---------------- dec1: K=64, 4 taps, M=512 (4 chunks) ----------------
            for t8 in range(8):
                oh0 = 8 * t8
                for mch in range(4):
                    p = psum.tile([128, 512], F32, tag="mm")
                    for kh in range(2):
                        nc.tensor.matmul(
                            p[:], wd1[:, kh * 512 + mch * 128: kh * 512 + (mch + 1) * 128],
                            zqv[:, oh0 + kh: oh0 + kh + 8, 0:64],
                            start=(kh == 0), stop=(kh == 1))
                    nc.scalar.activation(y1[mch][:, oh0 * 64: (oh0 + 8) * 64], p[:], GELU)

            if DEBUG and img == 0:
                nc.sync.dma_start(DY1, y1[0][:])
            if STAGES < 7:
                continue
            # ---------------- dec2: K=512 (4 slabs), 4 taps, M=256 (2 chunks) ----------------
            y1v = [y1[s][:].rearrange("p (r c) -> p r c", r=64, c=64) for s in range(4)]
            y2v = [y2[s][:].rearrange("p (r c) -> p r c", r=65, c=68) for s in range(2)]
            for t8 in range(8):
                oh0 = 8 * t8
                nr = 8 if t8 < 7 else 7
                for mch in range(2):
                    p = psum.tile([128, 504], F32, tag="mm")
                    first = True
                    for tap in range(4):
                        kh, kw = tap // 2, tap % 2
                        for s in range(4):
                            nc.tensor.matmul(
                                p[:, :nr * 63],
                                wd2[:, (tap * 4 + s) * 256 + mch * 128: (tap * 4 + s) * 256 + (mch + 1) * 128],
                                y1v[s][:, oh0 + kh: oh0 + kh + nr, kw: kw + 63],
                                start=first, stop=(tap == 3 and s == 3))
                            first = False
                    nc.scalar.activation(
                        y2v[mch][:, oh0 + 1: oh0 + 1 + nr, 1:64],
                        p[:, :nr * 63].rearrange("p (r c) -> p r c", r=nr, c=63), GELU)

            if DEBUG and img == 0:
                nc.sync.dma_start(DY2, y2[0][:])
            if STAGES < 8:
                continue
            # ---------------- dec3: K=256 (2 slabs), 4 taps, M=256 (2 chunks) ----------------
            for t8 in range(8):
                oh0 = 8 * t8
                for mch in range(2):
                    p = psum.tile([128, 512], F32, tag="mm")
                    first = True
                    for tap in range(4):
                        kh, kw = tap // 2, tap % 2
                        for s in range(2):
                            nc.tensor.matmul(
                                p[:],
                                wd3[:, (tap * 2 + s) * 256 + mch * 128: (tap * 2 + s) * 256 + (mch + 1) * 128],
                                y2v[s][:, oh0 + kh: oh0 + kh + 8, kw: kw + 64],
                                start=first, stop=(tap == 3 and s == 1))
                            first = False
                    xo = outp.tile([128, 512], F32, tag="xo")
                    nc.scalar.activation(xo[:], p[:], COPY)
                    nc.sync.dma_start(XH[img, mch * 128:(mch + 1) * 128, oh0 * 64:(oh0 + 8) * 64], xo[:])

        nc.sync.dma_start(MACC, macc[:])
        nc.sync.dma_start(ZACC, zacc[:])

    nc.compile()
    return nc

# memset y2 pads once per image is needed: y2 pad cols/rows written never read?
# dec3 reads padded region -> must be zero. y2 buffers are reused across images;
# interior is fully overwritten each image, pads stay zero from a single memset.


def _setup_profiling():
    """Shim antenv.axon_hooks (absent on this image) with the boot module's
    ctypes NTFF hook, and neuter the artifact upload."""
    try:
        import types
        import concourse.bass_utils as bu
        from trn_agent_boot.trn_boot import _ntff_profile_via_ctypes
        import antenv
        if "antenv.axon_hooks" not in sys.modules:
            hook = _ntff_profile_via_ctypes("/opt/axon/libaxon_pjrt.so")
            if hook is None:
                return False
            mod = types.ModuleType("antenv.axon_hooks")
            mod.get_axon_ntff_profile_hook = lambda: hook
            mod.set_axon_ntff_profile_hook = lambda h: None
            sys.modules["antenv.axon_hooks"] = mod
            antenv.axon_hooks = mod
        bu.upload_artifacts = lambda tmpdir: "local://" + str(tmpdir)
        return True
    except Exception as e:  # pragma: no cover
        print(f"profiling setup failed: {e}", flush=True)
        return False


# ---------------------------------------------------------------- host packing
def _pack_weights(enc_w1, enc_w2, enc_w3, dec_w1, dec_w2, dec_w3, codes):
    bf = ml_dtypes.bfloat16
    w1im = enc_w1.transpose(2, 3, 1, 0).reshape(75, 64).astype(np.float32)
    w1f = np.zeros((128, 128), np.float32)
    w1f[:75, 0:64] = w1im
    w1f[:75, 64:128] = w1im
    w1h = w1f.astype(bf)
    w1l = (w1f - w1h.astype(np.float32)).astype(bf)
    w1p = np.stack([w1h, w1l], 0)

    w2p = np.zeros((2, 5, 128, 64), np.float32)
    for g in range(2):
        for kw in range(5):
            w2p[g, kw, 0:64] = enc_w2[:, :, 2 * g, kw].T
            w2p[g, kw, 64:128] = enc_w2[:, :, 2 * g + 1, kw].T
    w2p4 = np.stack([enc_w2[:, :, 4, kw].T for kw in range(5)], 0).astype(np.float32)
    w3p = np.stack([enc_w3[:, :, t // 2, t % 2].T for t in range(4)], 0).astype(np.float32)

    # dec1 kw-pair: lhsT[kh] rows 0:64 = w(kh,kw=0), rows 64:128 = w(kh,kw=1)
    wd1 = np.stack(
        [np.concatenate([dec_w1[:, :, kh, 0].T, dec_w1[:, :, kh, 1].T], axis=0)
         for kh in range(2)], 0).astype(bf)
    wd2 = np.stack(
        [np.stack([dec_w2[:, 128 * s:128 * (s + 1), t // 2, t % 2].T for s in range(4)], 0)
         for t in range(4)], 0).astype(bf)
    wd3 = np.stack(
        [np.stack([dec_w3[:, 128 * s:128 * (s + 1), t // 2, t % 2].T for s in range(2)], 0)
         for t in range(4)], 0).astype(bf)

    codes2 = np.concatenate(
        [(2.0 * codes.T), -(codes.astype(np.float64) ** 2).sum(1)[None, :]],
        axis=0).astype(np.float32)                             # [65, 24]
    ctbl = np.concatenate([codes, codes], axis=1).astype(np.float32)   # [24, 128] dup
    return dict(w1p=w1p, w2p=w2p, w2p4=w2p4, w3p=w3p, wd1=wd1, wd2=wd2, wd3=wd3,
                codes2=codes2, ctbl=ctbl)


def _im2row(x4):
    """x4: [4, 3, 256, 256] fp32 -> [75, 4*16384] fp32 (conv1 im2row, pad 2 stride 2)."""
    xp = np.pad(x4, ((0, 0), (0, 0), (2, 2), (2, 2)))
    sl = np.empty((25, 4, 3, 128, 128), np.float32)
    for kh in range(5):
        for kw in range(5):
            sl[kh * 5 + kw] = xp[:, :, kh:kh + 256:2, kw:kw + 256:2]
    import ml_dtypes as _md
    out = np.zeros((128, 4 * 16384), np.float32)
    out[:75] = sl.transpose(0, 2, 1, 3, 4).reshape(75, 4 * 16384)
    hi = out.astype(_md.bfloat16)
    lo = (out - hi.astype(np.float32)).astype(_md.bfloat16)
    return np.stack([hi, lo], 0)


# ---------------------------------------------------------------- entry point
def kernel(x, enc_w1, enc_w2, enc_w3, dec_w1, dec_w2, dec_w3, codes, ema_count):
    x = np.asarray(x, np.float32)
    wpack = _pack_weights(np.asarray(enc_w1), np.asarray(enc_w2), np.asarray(enc_w3),
                          np.asarray(dec_w1), np.asarray(dec_w2), np.asarray(dec_w3),
                          np.asarray(codes))

    key = (STAGES, DEBUG, NOZ2)
    if key not in _CACHE:
        _CACHE[key] = build_nc()
    nc = _CACHE[key]

    in_maps = []
    for core in range(NCORES):
        m = dict(wpack)
        m["im2r"] = _im2row(x[core * P_IMG:(core + 1) * P_IMG])
        in_maps.append(m)

    trace = os.environ.get("KERNEL_PROFILE", "0") == "1"
    if trace:
        trace = _setup_profiling()
    res = run_bass_kernel_spmd(nc, in_maps, list(range(NCORES)), trace=trace)
    _CACHE["res"] = res
    if trace and res.exec_time_ns is not None:
        print(f"HW exec time: {res.exec_time_ns} ns", flush=True)
        _CACHE["exec_time_ns"] = res.exec_time_ns

    B, C, H, W = 32, 64, 63, 63
    x_hat = np.empty((32, 256, 64, 64), np.float32)
    idx_full = np.empty((32, 3969), np.int64)
    z2_sum = 0.0
    smax_sum = 0.0
    for core in range(NCORES):
        r = res.results[core]
        x_hat[core * P_IMG:(core + 1) * P_IMG] = r["xh"].reshape(P_IMG, 256, 64, 64)
        idx_full[core * P_IMG:(core + 1) * P_IMG] = r["idxo"].astype(np.int64)
        z2_sum += float(r["zacc_o"].astype(np.float64).sum())
        smax_sum += float(r["macc_o"].astype(np.float64).sum())

    ema = np.asarray(ema_count, np.float64)
    probs = (ema / ema.sum()).astype(np.float32)
    z_probs = probs[idx_full].reshape(32, 63, 63).astype(np.float32)

    vq_loss = np.float32(0.25 * (z2_sum - smax_sum) / (B * C * H * W))
    return x_hat, z_probs, vq_loss
